# revision 23
# baseline (speedup 1.0000x reference)
"""AnchorGNN forward on 8 TRN2 NeuronCores (Bass/Tile), nodes sharded 8-way.

Self-contained: hardcodes all shapes from the problem spec.
kernel(**inputs) takes the full unsharded inputs and returns
(pred [N,2], z [N,128], anchor_vec [1,A,128], anchor_logits [A,2]).

Layout strategy: activations live feature-major ([128 feat partitions, nodes
free]) so every matmul's stationary operand is a natural weight tile and
per-feature biases are per-partition scalars. The anchor-graph learner (the
dominant cost) runs the relu as a dual-op tensor_scalar on DVE (bf16, 4x
mode) and contracts on the PE with per-anchor column-masked stationary tiles
so all 32 anchors accumulate into one [32, chunk] PSUM tile. The node-dim
reductions (colsum, adj.T @ support) are computed as local partials and
combined with one tiny [32,129] AllReduce per hop; node_norm/anchor_norm are
never materialized (their scalings are factored onto the tiny side of each
product).
"""
import numpy as np

import concourse.bass as bass
import concourse.mybir as mybir
import concourse.tile as tile
from concourse.bass_utils import run_bass_kernel_spmd
from concourse.masks import make_identity

F32 = mybir.dt.float32
BF16 = mybir.dt.bfloat16
AF = mybir.ActivationFunctionType
ALU = mybir.AluOpType

N, NFEAT, NHID, A, P, HOPS = 50000, 256, 128, 32, 4, 2
NCORES = 8
NS = N // NCORES         # 6250 nodes per core
NSP = 6272               # padded to 49*128
NCH = NSP // 128         # 49 chunks of 128 nodes
EPS = 1e-12
LN_EPS = 1e-5

# matmul free-dim chunking of NSP (PSUM bank = 512 f32)
MM = []
_off = 0
for _w in [512] * 12 + [128]:
    MM.append((_off, _w))
    _off += _w
# learner super-chunks (relu granularity)
SUPS = []
_off = 0
for _w in [1024] * 6 + [128]:
    SUPS.append((_off, _w))
    _off += _w

# anchor -> (col-group, mask position) for 3-way PE column tiling
GOFF = [0, 11, 22]          # first anchor of each group
GSIZ = [11, 11, 10]
PR = 96                     # permuted anchor rows (3 groups x 32)


def j_to_gm(j):
    g = 0 if j < 11 else (1 if j < 22 else 2)
    return g, j - GOFF[g]


_MAX_WAITS = 1


_SPLIT_ENGINES = {
    mybir.EngineType.PE, mybir.EngineType.Activation, mybir.EngineType.DVE,
    mybir.EngineType.Pool, mybir.EngineType.SP,
}


def _split_waits(nc):
    """This walrus build rejects >1 sync-wait per TPB instruction; hoist extras
    onto preceding same-engine NoOps (streams are in-order, so equivalent).
    DMA-queue instructions lower via DGE rings and are left untouched."""
    ctr = [0]

    def fresh_nop(engine, waits):
        ctr[0] += 1
        nop = mybir.InstNoOp(name=f"I-waitsplit-{ctr[0]}", ins=[], outs=[])
        nop.engine = engine
        nop.sync_info = mybir.SyncInfo(on_wait=list(waits), on_update=[])
        return nop

    for fn in nc.m.functions:
        for blk in fn.blocks:
            new_insts = []
            for inst in blk.instructions:
                si = getattr(inst, "sync_info", None)
                eng = getattr(inst, "engine", None)
                if (
                    si is not None
                    and eng in _SPLIT_ENGINES
                    and si.on_wait is not None
                    and len(si.on_wait) > _MAX_WAITS
                ):
                    waits = list(si.on_wait)
                    extra, keep = waits[:-_MAX_WAITS], waits[-_MAX_WAITS:]
                    for i in range(0, len(extra), _MAX_WAITS):
                        new_insts.append(fresh_nop(eng, extra[i : i + _MAX_WAITS]))
                    inst.sync_info = mybir.SyncInfo(
                        on_wait=keep, on_update=list(si.on_update or [])
                    )
                new_insts.append(inst)
            blk.instructions[:] = new_insts
    return nc


def build_nc(split_waits=True):
    nc = bass.Bass("TRN2", target_bir_lowering=False, debug=False, num_devices=NCORES)

    def inp(name, shape):
        return nc.dram_tensor(name, shape, F32, kind="ExternalInput").ap()

    x_d = inp("x", [NSP, NFEAT])
    im_w1_d = inp("im_w1", [NFEAT, NHID])
    im_b1_d = inp("im_b1", [1, NHID])
    im_w2_d = inp("im_w2", [NHID, NHID])
    im_b2_d = inp("im_b2", [1, NHID])
    anchors_d = inp("anchors", [A, NHID])
    ml_w1_d = inp("ml_w1", [P * 2 * NHID, NHID])   # host-reshaped
    ml_b1_d = inp("ml_b1", [P, NHID])
    ml_w2_d = inp("ml_w2", [P, NHID])
    ml_b2_d = inp("ml_b2", [1, P])
    enc_w_d = inp("enc_w", [HOPS * NHID, NHID])    # host-reshaped
    enc_b_d = inp("enc_b", [HOPS, NHID])
    ln_g_d = inp("ln_g", [1, NHID])
    ln_b_d = inp("ln_b", [1, NHID])
    cls_w1_d = inp("cls_w1", [NHID, NHID])
    cls_b1_d = inp("cls_b1", [1, NHID])
    prelu_a_d = inp("prelu_a", [1, NHID])
    cls_w2_d = inp("cls_w2", [NHID, 2])
    cls_b2_d = inp("cls_b2", [1, 2])

    z_d = nc.dram_tensor("z", [NSP, NHID], F32, kind="ExternalOutput").ap()
    pred_d = nc.dram_tensor("pred", [NSP, 2], F32, kind="ExternalOutput").ap()
    alog_d = nc.dram_tensor("alog", [A, 2], F32, kind="ExternalOutput").ap()

    with tile.TileContext(nc) as tc:
        with (
            tc.tile_pool(name="wts", bufs=1) as wp,
            tc.tile_pool(name="bigs", bufs=1) as bigp,
            tc.tile_pool(name="hxp", bufs=2) as hxp,
            tc.tile_pool(name="xin", bufs=3) as xp,
            tc.tile_pool(name="work", bufs=3) as wk,
            tc.tile_pool(name="tmpl", bufs=4) as tmpl,
            tc.tile_pool(name="psmm", bufs=3, space="PSUM") as psmm,
            tc.tile_pool(name="psat", bufs=2, space="PSUM") as psat,
            tc.tile_pool(name="dram", bufs=1, space="DRAM") as drp,
        ):
            # ============ weights & small derived tiles ============
            ident = wp.tile([128, 128], F32)
            make_identity(nc, ident)
            identb = wp.tile([128, 128], BF16)
            nc.vector.tensor_copy(identb[:, :], ident[:, :])
            ones_r = wp.tile([1, 128], F32)
            nc.vector.memset(ones_r[:, :], 1.0)
            ones_rb = wp.tile([1, 128], BF16)
            nc.vector.memset(ones_rb[:, :], 1.0)

            def load_f32(name, src_ap, p, f):
                t = wp.tile([p, f], F32, name=name)
                nc.sync.dma_start(out=t[:, :], in_=src_ap)
                return t

            def cast_bf16(name, src, p, f):
                t = wp.tile([p, f], BF16, name=name)
                nc.vector.tensor_copy(t[:, :], src[:, :])
                return t

            def load_bf16(name, src_ap, p, f):
                return cast_bf16(name, load_f32(name + "_f", src_ap, p, f), p, f)

            im_w1b = [
                load_bf16(f"im_w1b{k}", im_w1_d[k * 128 : (k + 1) * 128, :], 128, NHID)
                for k in range(2)
            ]
            im_w2b = load_bf16("im_w2b", im_w2_d[:, :], 128, NHID)
            Wxb = [
                load_bf16(f"Wxb{p}", ml_w1_d[p * 256 : p * 256 + 128, :], 128, NHID)
                for p in range(P)
            ]
            Wab = [
                load_bf16(f"Wab{p}", ml_w1_d[p * 256 + 128 : p * 256 + 256, :], 128, NHID)
                for p in range(P)
            ]
            enc_wb = [
                load_bf16(f"enc_wb{i}", enc_w_d[i * 128 : (i + 1) * 128, :], 128, NHID)
                for i in range(HOPS)
            ]
            cls_w1b = load_bf16("cls_w1b", cls_w1_d[:, :], 128, NHID)
            cls_w2b = load_bf16("cls_w2b", cls_w2_d[:, :], 128, 2)

            # column-vector biases via outer product with ones ([1,1] rhs)
            def bias_col(name, src_ap, length):
                row = load_f32(name + "_r", src_ap, 1, length)
                ps = psmm.tile([length, 1], F32, name=name + "_ps", tag="mm")
                nc.tensor.matmul(ps[:, :], row[:, :], ones_r[0:1, 0:1],
                                 start=True, stop=True)
                col = wp.tile([length, 1], F32, name=name)
                nc.scalar.copy(col[:, :], ps[:, :])
                return col

            im_b1c = bias_col("im_b1c", im_b1_d[:, :], NHID)
            im_b2c = bias_col("im_b2c", im_b2_d[:, :], NHID)
            enc_bc = [
                bias_col(f"enc_bc{i}", enc_b_d[i : i + 1, :], NHID) for i in range(HOPS)
            ]
            cls_b1c = bias_col("cls_b1c", cls_b1_d[:, :], NHID)
            prelu_ac = bias_col("prelu_ac", prelu_a_d[:, :], NHID)
            cls_b2c = bias_col("cls_b2c", cls_b2_d[:, :], 2)
            neg_b1c = wp.tile([NHID, 1], F32, name="neg_b1c")
            nc.vector.tensor_scalar(out=neg_b1c[:, :], in0=cls_b1c[:, :],
                                    scalar1=-1.0, scalar2=None, op0=ALU.mult)

            # LN gamma/beta broadcast tiles [128 nodes, 128 feat]
            def bcast_row(name, src_ap):
                row = load_f32(name + "_r", src_ap, 1, NHID)
                ps = psmm.tile([128, NHID], F32, name=name + "_ps", tag="mm")
                nc.tensor.matmul(ps[:, :], ones_r[:, :], row[:, :],
                                 start=True, stop=True)
                t = wp.tile([128, NHID], BF16, name=name)
                nc.scalar.copy(t[:, :], ps[:, :])
                return t

            gBb = bcast_row("gBb", ln_g_d[:, :])
            bBb = bcast_row("bBb", ln_b_d[:, :])

            # anchorsT [128, 32] bf16
            anchors_f = load_f32("anchors_f", anchors_d[:, :], A, NHID)
            anchors_fb = cast_bf16("anchors_fb", anchors_f, A, NHID)
            aT_ps = psmm.tile([NHID, A], BF16, name="aT_ps", tag="mm")
            nc.tensor.transpose(aT_ps[:, :], anchors_fb[:, :], identb[0:A, 0:A])
            anchorsTb = wp.tile([NHID, A], BF16, name="anchorsTb")
            nc.scalar.copy(anchorsTb[:, :], aT_ps[:, :])

            # ml_b1T [128, P] f32 ; ml_w2T [128, P]
            ml_b1_f = load_f32("ml_b1_f", ml_b1_d[:, :], P, NHID)
            mb1_ps = psmm.tile([NHID, P], F32, name="mb1_ps", tag="mm")
            nc.tensor.transpose(mb1_ps[:, :], ml_b1_f[:, :], ident[0:P, 0:P])
            ml_b1T = wp.tile([NHID, P], F32, name="ml_b1T")
            nc.scalar.copy(ml_b1T[:, :], mb1_ps[:, :])

            ml_w2_f = load_f32("ml_w2_f", ml_w2_d[:, :], P, NHID)
            mw2_ps = psmm.tile([NHID, P], F32, name="mw2_ps", tag="mm")
            nc.tensor.transpose(mw2_ps[:, :], ml_w2_f[:, :], ident[0:P, 0:P])
            ml_w2Tb = wp.tile([NHID, P], BF16, name="ml_w2Tb")
            nc.scalar.copy(ml_w2Tb[:, :], mw2_ps[:, :])

            # masked stationary tiles: per (p, mask-pos m), [128, 32] with
            # w2_p in column m; shared by the 3 column-group tiles.
            w2m = wp.tile([128, P * 11 * 32], BF16, name="w2m")
            nc.vector.memset(w2m[:, :], 0.0)
            for p in range(P):
                for m in range(11):
                    cidx = (p * 11 + m) * 32 + m
                    nc.vector.tensor_copy(
                        w2m[:, cidx : cidx + 1], ml_w2Tb[:, p : p + 1]
                    )

            # b2 per-head bias columns on 32 partitions: [32, P]
            mlb2_row = load_f32("mlb2_row", ml_b2_d[:, :], 1, P)
            b2_ps = psmm.tile([128, P], F32, name="b2_ps", tag="mm")
            nc.tensor.matmul(b2_ps[:, :], ones_r[0:1, :], mlb2_row[:, :],
                             start=True, stop=True)
            b2col = wp.tile([128, P], F32, name="b2col")
            nc.scalar.copy(b2col[:, :], b2_ps[:, :])

            # perm-row validity mask [PR, 1] (1.0 on rows holding an anchor)
            selr = wp.tile([1, PR], F32, name="selr")
            nc.vector.memset(selr[:, :], 0.0)
            for g in range(3):
                nc.vector.memset(selr[0:1, 32 * g : 32 * g + GSIZ[g]], 1.0)
            selrb = wp.tile([1, PR], BF16, name="selrb")
            nc.vector.tensor_copy(selrb[:, :], selr[:, :])
            mc_ps = psmm.tile([PR, 1], F32, name="mc_ps", tag="mm")
            nc.tensor.matmul(mc_ps[:, :], selr[:, :], ones_r[0:1, 0:1],
                             start=True, stop=True)
            mask_col = wp.tile([PR, 1], F32, name="mask_col")
            nc.scalar.copy(mask_col[:, :], mc_ps[:, :])
            mrb_ps = psmm.tile([128, PR], F32, name="mrb_ps", tag="mm")
            nc.tensor.matmul(mrb_ps[:, :], ones_rb[:, :], selrb[:, :],
                             start=True, stop=True)
            maskRowB = wp.tile([128, PR], BF16, name="maskRowB")
            nc.scalar.copy(maskRowB[:, :], mrb_ps[:, :])

            # ============ input mapper: x -> hT [128, NSP] bf16 ============
            hT = bigp.tile([NHID, NSP], BF16, name="hT")
            for (off, w) in MM:
                nsub = w // 128
                xas = []
                for sNum in range(nsub):
                    c = off // 128 + sNum
                    xa = xp.tile([128, NFEAT], F32, name=f"xa{c}", tag=f"xa{sNum}")
                    nc.sync.dma_start(out=xa[:, :],
                                      in_=x_d[c * 128 : (c + 1) * 128, :])
                    xab = xp.tile([128, NFEAT], BF16, name=f"xab{c}", tag=f"xab{sNum}")
                    nc.vector.tensor_copy(xab[:, :], xa[:, :])
                    xas.append(xab)
                # transposed x: [256 feat(2 ktiles), w nodes]
                xt_ps = [
                    psmm.tile([128, w], BF16, name=f"xt_ps{off}_{k}", tag="mm")
                    for k in range(2)
                ]
                for k in range(2):
                    for sNum in range(nsub):
                        nc.tensor.transpose(
                            xt_ps[k][:, sNum * 128 : (sNum + 1) * 128],
                            xas[sNum][:, k * 128 : (k + 1) * 128],
                            identb[:, :],
                        )
                xtb = wk.tile([128, 2 * w], BF16, name=f"xtb{off}", tag="xtb")
                nc.vector.tensor_copy(xtb[:, 0:w], xt_ps[0][:, :])
                nc.vector.tensor_copy(xtb[:, w : 2 * w], xt_ps[1][:, :])
                h1_ps = psmm.tile([NHID, w], F32, name=f"h1_ps{off}", tag="mm")
                for k in range(2):
                    nc.tensor.matmul(
                        h1_ps[:, :], im_w1b[k][:, :], xtb[:, k * w : (k + 1) * w],
                        start=(k == 0), stop=(k == 1),
                    )
                r1 = wk.tile([NHID, w], BF16, name=f"r1{off}", tag="r1")
                nc.vector.tensor_scalar(
                    out=r1[:, :], in0=h1_ps[:, :], scalar1=im_b1c[:, :],
                    scalar2=0.0, op0=ALU.add, op1=ALU.max,
                )
                h2_ps = psmm.tile([NHID, w], F32, name=f"h2_ps{off}", tag="mm")
                nc.tensor.matmul(h2_ps[:, :], im_w2b[:, :], r1[:, :],
                                 start=True, stop=True)
                nc.vector.tensor_scalar(
                    out=hT[:, off : off + w], in0=h2_ps[:, :],
                    scalar1=im_b2c[:, :], scalar2=None, op0=ALU.add,
                )

            # hop-0 support depends only on hT; emit early so its PE/ACT work
            # overlaps the learner instead of the serial tail.
            def support_nm_fwd(src_T, w_b, name, s_nm=None, mm_list=None):
                if s_nm is None:
                    s_nm = bigp.tile([128, NSP], BF16, name=name, tag="s_nm")
                for (off, w) in (mm_list if mm_list is not None else MM):
                    sp_ps = psmm.tile([NHID, w], F32, name=f"{name}_ps{off}", tag="mm")
                    nc.tensor.matmul(sp_ps[:, :], w_b[:, :], src_T[:, off : off + w],
                                     start=True, stop=True)
                    sT = wk.tile([NHID, w], BF16, name=f"{name}_sT{off}", tag="sT")
                    nc.scalar.copy(sT[:, :], sp_ps[:, :])
                    tr_ps = psmm.tile([128, w], BF16, name=f"{name}_tr{off}", tag="mm")
                    nsub = w // 128
                    for sNum in range(nsub):
                        nc.tensor.transpose(
                            tr_ps[:, sNum * 128 : (sNum + 1) * 128],
                            sT[:, sNum * 128 : (sNum + 1) * 128],
                            identb[:, :],
                        )
                    nc.scalar.copy(s_nm[:, off : off + w], tr_ps[:, :])
                return s_nm

            s0_nm = bigp.tile([128, NSP], BF16, name="s0_nm", tag="s_nm")
            S0_CHUNKS = [MM[0:3], MM[3:6], MM[6:9], MM[9:13]]

            # ============ learner: adj_perm [PR, NSP] bf16 ============
            # (row 32g+m holds anchor GOFF[g]+m; other rows are sigmoid(b2))
            adjT = bigp.tile([PR, NSP], BF16, name="adjT")
            for p in range(P):
                support_nm_fwd(hT, enc_wb[0], f"s0nm{p}", s_nm=s0_nm,
                               mm_list=S0_CHUNKS[p])
                # c_pT = anchors @ Wa_p (transposed) + ml_b1[p]
                ha_ps = psmm.tile([NHID, A], F32, name=f"ha_ps{p}", tag="mm")
                nc.tensor.matmul(ha_ps[:, :], Wab[p][:, :], anchorsTb[:, :],
                                 start=True, stop=True)
                c_pT = wk.tile([NHID, A], F32, name=f"c_pT{p}", tag="cpT")
                nc.vector.tensor_scalar(
                    out=c_pT[:, :], in0=ha_ps[:, :],
                    scalar1=ml_b1T[:, p : p + 1], scalar2=None, op0=ALU.add,
                )
                # hx_p = h @ Wx_p (transposed), bf16
                hxb = hxp.tile([NHID, NSP], BF16, name=f"hxb{p}", tag="hxb")
                for (off, w) in MM:
                    hx_ps = psmm.tile([NHID, w], F32, name=f"hx_ps{p}_{off}", tag="mm")
                    nc.tensor.matmul(hx_ps[:, :], Wxb[p][:, :], hT[:, off : off + w],
                                     start=True, stop=True)
                    nc.scalar.copy(hxb[:, off : off + w], hx_ps[:, :])
                # per super-chunk: relu per anchor, masked matmuls, sigmoid
                for (soff, sw) in SUPS:
                    nbank = (sw + 511) // 512
                    pss = [
                        psat.tile([PR, min(512, sw - k * 512)], F32,
                                  name=f"at{p}_{soff}_{k}", tag=f"at{k}")
                        for k in range(nbank)
                    ]
                    jseq = []
                    for m in range(11):
                        for g in range(3):
                            if m < GSIZ[g]:
                                jseq.append((g, m, GOFF[g] + m))
                    for (g, m, j) in jseq:
                        t = tmpl.tile([128, sw], BF16, name=f"t{p}{soff}{j}", tag="t")
                        nc.vector.tensor_scalar(
                            out=t[:, :], in0=hxb[:, soff : soff + sw],
                            scalar1=c_pT[:, j : j + 1], scalar2=0.0,
                            op0=ALU.add, op1=ALU.max,
                        )
                        mcol = (p * 11 + m) * 32
                        for k in range(nbank):
                            kw = min(512, sw - k * 512)
                            nc.tensor.matmul(
                                pss[k][32 * g : 32 * (g + 1), :],
                                w2m[:, mcol : mcol + 32],
                                t[:, k * 512 : k * 512 + kw],
                                start=(m == 0), stop=(m == GSIZ[g] - 1),
                                tile_position=(0, 32 * g),
                                skip_group_check=True,
                            )
                    for k in range(nbank):
                        kw = min(512, sw - k * 512)
                        dst = adjT[:, soff + k * 512 : soff + k * 512 + kw]
                        if p == 0:
                            nc.scalar.activation(
                                dst, pss[k][:, :], AF.Sigmoid,
                                bias=b2col[0:PR, p : p + 1],
                            )
                        else:
                            sg = wk.tile([PR, kw], BF16, name=f"sg{p}{soff}{k}", tag="sg")
                            nc.scalar.activation(
                                sg[:, :], pss[k][:, :], AF.Sigmoid,
                                bias=b2col[0:PR, p : p + 1],
                            )
                            nc.vector.tensor_tensor(
                                out=dst, in0=dst, in1=sg[:, :], op=ALU.add
                            )

            # zero the padded node columns so node-reductions are exact
            nc.vector.memset(adjT[:, NS:NSP], 0.0)

            # ============ norms prep ============
            colsum = bigp.tile([PR, 1], F32, name="colsum")
            nc.vector.tensor_reduce(colsum[:, :], adjT[:, :],
                                    mybir.AxisListType.X, ALU.add)

            # adj_nm [128, NCH*32] bf16 : node-major chunks of adjT
            adj_nm = hxp.tile([128, NCH * PR], BF16, name="adj_nm", tag="hxb")
            for (off, w) in MM:
                nsub = w // 128
                tr_ps = psmm.tile([128, nsub * PR], BF16,
                                  name=f"adjnm_ps{off}", tag="mm")
                for s in range(nsub):
                    c = off // 128 + s
                    nc.tensor.transpose(
                        tr_ps[:, s * PR : (s + 1) * PR],
                        adjT[:, c * 128 : (c + 1) * 128],
                        identb[0:PR, 0:PR],
                    )
                nc.scalar.copy(
                    adj_nm[:, (off // 128) * PR : (off // 128 + nsub) * PR],
                    tr_ps[:, :],
                )

            # rowsum per node -> rinv broadcast tile [128, NSP] bf16
            rs_nm = bigp.tile([128, NCH], F32, name="rs_nm")
            for c in range(NCH):
                amk = wk.tile([128, PR], BF16, name=f"amk{c}", tag="amk")
                nc.vector.tensor_tensor(
                    out=amk[:, :], in0=adj_nm[:, c * PR : (c + 1) * PR],
                    in1=maskRowB[:, :], op=ALU.mult,
                )
                nc.vector.tensor_reduce(
                    rs_nm[:, c : c + 1], amk[:, :],
                    mybir.AxisListType.X, ALU.add,
                )
            nc.vector.tensor_scalar(out=rs_nm[:, :], in0=rs_nm[:, :],
                                    scalar1=EPS, scalar2=None, op0=ALU.max)
            rinv_nm = bigp.tile([128, NCH], F32, name="rinv_nm")
            nc.vector.reciprocal(rinv_nm[:, :], rs_nm[:, :])
            # rearrange to free-major [1, NSP] via per-column PE transposes
            rinv_fm = bigp.tile([1, NSP], BF16, name="rinv_fm")
            for (off, w) in MM:
                nsub = w // 128
                rt_ps = psmm.tile([1, w], F32, name=f"rt_ps{off}", tag="mm")
                for s in range(nsub):
                    c = off // 128 + s
                    nc.tensor.transpose(
                        rt_ps[:, s * 128 : (s + 1) * 128],
                        rinv_nm[:, c : c + 1], ident[:, :],
                    )
                nc.scalar.copy(rinv_fm[0:1, off : off + w], rt_ps[:, :])
            rinvB = bigp.tile([128, NSP], BF16, name="rinvB")
            for (off, w) in MM:
                nsub = w // 128
                rb_ps = psmm.tile([128, w], F32, name=f"rb_ps{off}", tag="mm")
                for s in range(nsub):
                    nc.tensor.matmul(
                        rb_ps[:, s * 128 : (s + 1) * 128],
                        ones_rb[:, :],
                        rinv_fm[0:1, off + s * 128 : off + (s + 1) * 128],
                        start=True, stop=True,
                    )
                nc.scalar.copy(rinvB[:, off : off + w], rb_ps[:, :])

            # ============ GNN hops ============
            support_nm = support_nm_fwd

            def agg_raw(s_nm, name):
                """[PR, 128] PSUM partial of adj.T @ support (over local nodes)."""
                ps = psat.tile([PR, NHID], F32, name=name, tag="at0")
                for c in range(NCH):
                    nc.tensor.matmul(
                        ps[:, :], adj_nm[:, c * PR : (c + 1) * PR],
                        s_nm[:, c * 128 : (c + 1) * 128],
                        start=(c == 0), stop=(c == NCH - 1),
                    )
                return ps

            def hop_apply(agg_b, enc_bc_i, name):
                """xc_next.T = relu((agg.T @ adjT) * rinvB + enc_b)."""
                xcT = bigp.tile([NHID, NSP], BF16, name=name)
                for (off, w) in MM:
                    xp_ps = psmm.tile([NHID, w], F32, name=f"{name}_ps{off}", tag="mm")
                    nc.tensor.matmul(xp_ps[:, :], agg_b[:, :], adjT[:, off : off + w],
                                     start=True, stop=True)
                    xpre = wk.tile([NHID, w], BF16, name=f"{name}_pre{off}", tag="sT")
                    nc.scalar.copy(xpre[:, :], xp_ps[:, :])
                    xm = wk.tile([NHID, w], BF16, name=f"{name}_m{off}", tag="xm")
                    nc.vector.tensor_tensor(
                        out=xm[:, :], in0=xpre[:, :],
                        in1=rinvB[:, off : off + w], op=ALU.mult,
                    )
                    nc.vector.tensor_scalar(
                        out=xcT[:, off : off + w], in0=xm[:, :],
                        scalar1=enc_bc_i[:, :], scalar2=0.0,
                        op0=ALU.add, op1=ALU.max,
                    )
                return xcT

            # hop 0: fused AllReduce of [agg0_raw | colsum]
            agg0_ps = agg_raw(s0_nm, "agg0_ps")
            arbuf = bigp.tile([PR, 132], F32, name="arbuf")
            nc.scalar.copy(arbuf[:, 0:NHID], agg0_ps[:, :])
            nc.vector.tensor_copy(arbuf[:, NHID : NHID + 1], colsum[:, :])
            drin0 = drp.tile([PR, 129], F32, name="drin0")
            drout0 = drp.tile([PR, 129], F32, name="drout0")
            nc.sync.dma_start(out=drin0[:, :], in_=arbuf[:, 0:129])
            nc.gpsimd.collective_compute(
                "AllReduce", ALU.add,
                replica_groups=[list(range(NCORES))],
                ins=[drin0[:, :].opt()], outs=[drout0[:, :].opt()],
            )
            arres0 = bigp.tile([PR, 132], F32, name="arres0")
            nc.sync.dma_start(out=arres0[:, 0:129], in_=drout0[:, :])

            csc = bigp.tile([PR, 1], F32, name="csc")
            nc.vector.tensor_scalar(out=csc[:, :], in0=arres0[:, NHID : NHID + 1],
                                    scalar1=EPS, scalar2=None, op0=ALU.max)
            csinv = bigp.tile([PR, 1], F32, name="csinv")
            nc.vector.reciprocal(csinv[:, :], csc[:, :])
            # fold the perm-row validity mask into the scale
            nc.vector.tensor_scalar(out=csinv[:, :], in0=csinv[:, :],
                                    scalar1=mask_col[:, :], scalar2=None,
                                    op0=ALU.mult)
            agg0b = bigp.tile([PR, NHID], BF16, name="agg0b")
            nc.vector.tensor_scalar(out=agg0b[:, :], in0=arres0[:, 0:NHID],
                                    scalar1=csinv[:, :], scalar2=None, op0=ALU.mult)

            xc1T = hop_apply(agg0b, enc_bc[0], "xc1T")

            # hop 1
            s1_nm = support_nm(xc1T, enc_wb[1], "s1_nm")
            agg1_ps = agg_raw(s1_nm, "agg1_ps")
            arbuf1 = bigp.tile([PR, NHID], F32, name="arbuf1")
            nc.scalar.copy(arbuf1[:, :], agg1_ps[:, :])
            drin1 = drp.tile([PR, NHID], F32, name="drin1")
            drout1 = drp.tile([PR, NHID], F32, name="drout1")
            nc.sync.dma_start(out=drin1[:, :], in_=arbuf1[:, :])
            nc.gpsimd.collective_compute(
                "AllReduce", ALU.add,
                replica_groups=[list(range(NCORES))],
                ins=[drin1[:, :].opt()], outs=[drout1[:, :].opt()],
            )
            arres1 = bigp.tile([PR, NHID], F32, name="arres1")
            nc.sync.dma_start(out=arres1[:, :], in_=drout1[:, :])
            agg1b = bigp.tile([PR, NHID], BF16, name="agg1b")
            nc.vector.tensor_scalar(out=agg1b[:, :], in0=arres1[:, :],
                                    scalar1=csinv[:, :], scalar2=None, op0=ALU.mult)

            xc2T = hop_apply(agg1b, enc_bc[1], "xc2T")

            # ============ residual + LN (node-major) ============
            zTb = bigp.tile([NHID, NSP], BF16, name="zTb", tag="xc1T")
            for (off, w) in MM:
                nc.vector.tensor_tensor(
                    out=zTb[:, off : off + w], in0=xc2T[:, off : off + w],
                    in1=hT[:, off : off + w], op=ALU.add,
                )
            z_nm = bigp.tile([128, NSP], BF16, name="z_nm", tag="s_nm")
            for (off, w) in MM:
                tr_ps = psmm.tile([128, w], BF16, name=f"znm_ps{off}", tag="mm")
                nsub = w // 128
                for s in range(nsub):
                    nc.tensor.transpose(
                        tr_ps[:, s * 128 : (s + 1) * 128],
                        zTb[:, off + s * 128 : off + (s + 1) * 128],
                        identb[:, :],
                    )
                nc.scalar.copy(z_nm[:, off : off + w], tr_ps[:, :])

            mu_all = bigp.tile([128, NCH], F32, name="mu_all")
            sq_all = bigp.tile([128, NCH], F32, name="sq_all")
            for (off, w) in MM:
                nsub = w // 128
                c0 = off // 128
                zc3 = z_nm[:, off : off + w].rearrange("p (c f) -> p c f", f=128)
                nc.vector.tensor_reduce(mu_all[:, c0 : c0 + nsub], zc3,
                                        mybir.AxisListType.X, ALU.add)
                sqt = wk.tile([128, w], BF16, name=f"sqt{off}", tag="sqt", bufs=1)
                nc.vector.tensor_tensor(out=sqt[:, :], in0=z_nm[:, off : off + w],
                                        in1=z_nm[:, off : off + w], op=ALU.mult)
                nc.vector.tensor_reduce(
                    sq_all[:, c0 : c0 + nsub],
                    sqt[:, :].rearrange("p (c f) -> p c f", f=128),
                    mybir.AxisListType.X, ALU.add)
            nc.vector.tensor_scalar(out=mu_all[:, :], in0=mu_all[:, :],
                                    scalar1=1.0 / NHID, scalar2=None, op0=ALU.mult)
            nc.vector.tensor_scalar(out=sq_all[:, :], in0=sq_all[:, :],
                                    scalar1=1.0 / NHID, scalar2=None, op0=ALU.mult)
            msq = bigp.tile([128, NCH], F32, name="msq")
            nc.vector.tensor_tensor(out=msq[:, :], in0=mu_all[:, :],
                                    in1=mu_all[:, :], op=ALU.mult)
            var = bigp.tile([128, NCH], F32, name="var")
            nc.vector.tensor_tensor(out=var[:, :], in0=sq_all[:, :],
                                    in1=msq[:, :], op=ALU.subtract)
            stdt = bigp.tile([128, NCH], F32, name="stdt")
            lneps_c = wp.tile([128, 1], F32, name="lneps_c")
            nc.vector.memset(lneps_c[:, :], LN_EPS)
            nc.scalar.activation(stdt[:, :], var[:, :], AF.Sqrt, bias=lneps_c[:, :])
            rstd = bigp.tile([128, NCH], F32, name="rstd")
            nc.vector.reciprocal(rstd[:, :], stdt[:, :])

            znT = bigp.tile([NHID, NSP], BF16, name="znT", tag="rinvB")
            for (off, w) in MM:
                nsub = w // 128
                trz_ps = psmm.tile([128, w], F32, name=f"znt_ps{off}", tag="mm")
                for s in range(nsub):
                    c = off // 128 + s
                    zc = z_nm[:, c * 128 : (c + 1) * 128]
                    t1 = wk.tile([128, 128], BF16, name=f"lnt1_{c}", tag="lnt1")
                    nc.vector.tensor_scalar(
                        out=t1[:, :], in0=zc,
                        scalar1=mu_all[:, c : c + 1], scalar2=rstd[:, c : c + 1],
                        op0=ALU.subtract, op1=ALU.mult,
                    )
                    t2 = wk.tile([128, 128], BF16, name=f"lnt2_{c}", tag="lnt2")
                    nc.vector.tensor_tensor(out=t2[:, :], in0=t1[:, :],
                                            in1=gBb[:, :], op=ALU.mult)
                    znf = wk.tile([128, 128], F32, name=f"znf_{c}", tag="znf")
                    nc.vector.tensor_tensor(out=znf[:, :], in0=t2[:, :],
                                            in1=bBb[:, :], op=ALU.add)
                    if c * 128 < NS:
                        hi = min(128, NS - c * 128)
                        nc.sync.dma_start(
                            out=z_d[c * 128 : c * 128 + hi, :], in_=znf[0:hi, :]
                        )
                    nc.tensor.transpose(
                        trz_ps[:, s * 128 : (s + 1) * 128], znf[:, :], ident[:, :]
                    )
                if True:
                    nc.scalar.copy(znT[:, off : off + w], trz_ps[:, :])

            # ============ classifier ============
            for (off, w) in MM:
                t1_ps = psmm.tile([NHID, w], F32, name=f"cls_ps{off}", tag="mm")
                nc.tensor.matmul(t1_ps[:, :], cls_w1b[:, :], znT[:, off : off + w],
                                 start=True, stop=True)
                r1p = wk.tile([NHID, w], BF16, name=f"r1p{off}", tag="sT")
                nc.scalar.activation(r1p[:, :], t1_ps[:, :], AF.Relu,
                                     bias=cls_b1c[:, :])
                r2p = wk.tile([NHID, w], BF16, name=f"r2p{off}", tag="xm")
                nc.scalar.activation(r2p[:, :], t1_ps[:, :], AF.Relu,
                                     bias=neg_b1c[:, :], scale=-1.0)
                r2s = wk.tile([NHID, w], BF16, name=f"r2s{off}", tag="r2s")
                nc.vector.tensor_scalar(out=r2s[:, :], in0=r2p[:, :],
                                        scalar1=prelu_ac[:, :], scalar2=None,
                                        op0=ALU.mult)
                t1b = wk.tile([NHID, w], BF16, name=f"t1b{off}", tag="t1b")
                nc.vector.tensor_tensor(out=t1b[:, :], in0=r1p[:, :],
                                        in1=r2s[:, :], op=ALU.subtract)
                pr_ps = psmm.tile([2, w], F32, name=f"pr_ps{off}", tag="mm")
                nc.tensor.matmul(pr_ps[:, :], cls_w2b[:, :], t1b[:, :],
                                 start=True, stop=True)
                predc = wk.tile([2, w], F32, name=f"predc{off}", tag="predc")
                nc.scalar.activation(predc[:, :], pr_ps[:, :], AF.Identity,
                                     bias=cls_b2c[:, :])
                nsub = w // 128
                pt_ps = psmm.tile([128, nsub * 2], F32, name=f"pt_ps{off}", tag="mm")
                for s in range(nsub):
                    nc.tensor.transpose(
                        pt_ps[:, s * 2 : (s + 1) * 2],
                        predc[:, s * 128 : (s + 1) * 128],
                        ident[0:2, 0:2],
                    )
                predn = wk.tile([128, nsub * 2], F32, name=f"predn{off}", tag="predn")
                nc.scalar.copy(predn[:, :], pt_ps[:, :])
                for s in range(nsub):
                    c = off // 128 + s
                    if c * 128 < NS:
                        hi = min(128, NS - c * 128)
                        nc.sync.dma_start(
                            out=pred_d[c * 128 : c * 128 + hi, :],
                            in_=predn[0:hi, s * 2 : (s + 1) * 2],
                        )

            # anchors through the classifier
            a1_ps = psmm.tile([NHID, A], F32, name="a1_ps", tag="mm")
            nc.tensor.matmul(a1_ps[:, :], cls_w1b[:, :], anchorsTb[:, :],
                             start=True, stop=True)
            ar1 = wk.tile([NHID, A], BF16, name="ar1", tag="cpT")
            nc.scalar.activation(ar1[:, :], a1_ps[:, :], AF.Relu,
                                 bias=cls_b1c[:, :])
            ar2 = wk.tile([NHID, A], BF16, name="ar2", tag="ar2")
            nc.scalar.activation(ar2[:, :], a1_ps[:, :], AF.Relu,
                                 bias=neg_b1c[:, :], scale=-1.0)
            ar2s = wk.tile([NHID, A], BF16, name="ar2s", tag="ar2s")
            nc.vector.tensor_scalar(out=ar2s[:, :], in0=ar2[:, :],
                                    scalar1=prelu_ac[:, :], scalar2=None,
                                    op0=ALU.mult)
            a1b = wk.tile([NHID, A], BF16, name="a1b", tag="a1b")
            nc.vector.tensor_tensor(out=a1b[:, :], in0=ar1[:, :],
                                    in1=ar2s[:, :], op=ALU.subtract)
            a2_ps = psmm.tile([2, A], F32, name="a2_ps", tag="mm")
            nc.tensor.matmul(a2_ps[:, :], cls_w2b[:, :], a1b[:, :],
                             start=True, stop=True)
            alsb = wk.tile([2, A], F32, name="alsb", tag="predc")
            nc.scalar.activation(alsb[:, :], a2_ps[:, :], AF.Identity,
                                 bias=cls_b2c[:, :])
            alt_ps = psmm.tile([A, 2], F32, name="alt_ps", tag="mm")
            nc.tensor.transpose(alt_ps[:, :], alsb[:, :], ident[0:2, 0:2])
            alog_sb = wk.tile([A, 2], F32, name="alog_sb", tag="alog")
            nc.scalar.copy(alog_sb[:, :], alt_ps[:, :])
            nc.sync.dma_start(out=alog_d[:, :], in_=alog_sb[:, :])

    if split_waits:
        _split_waits(nc)
    return nc


_NC = None


def kernel(**inputs):
    global _NC
    if _NC is None:
        _NC = build_nc()

    f32 = np.float32
    x = np.asarray(inputs["x"], f32)
    base = {
        "im_w1": np.asarray(inputs["im_w1"], f32),
        "im_b1": np.asarray(inputs["im_b1"], f32).reshape(1, NHID),
        "im_w2": np.asarray(inputs["im_w2"], f32),
        "im_b2": np.asarray(inputs["im_b2"], f32).reshape(1, NHID),
        "anchors": np.asarray(inputs["anchors"], f32),
        "ml_w1": np.asarray(inputs["ml_w1"], f32).reshape(P * 2 * NHID, NHID),
        "ml_b1": np.asarray(inputs["ml_b1"], f32),
        "ml_w2": np.asarray(inputs["ml_w2"], f32),
        "ml_b2": np.asarray(inputs["ml_b2"], f32).reshape(1, P),
        "enc_w": np.asarray(inputs["enc_w"], f32).reshape(HOPS * NHID, NHID),
        "enc_b": np.asarray(inputs["enc_b"], f32),
        "ln_g": np.asarray(inputs["ln_g"], f32).reshape(1, NHID),
        "ln_b": np.asarray(inputs["ln_b"], f32).reshape(1, NHID),
        "cls_w1": np.asarray(inputs["cls_w1"], f32),
        "cls_b1": np.asarray(inputs["cls_b1"], f32).reshape(1, NHID),
        "prelu_a": np.asarray(inputs["prelu_a"], f32).reshape(1, NHID),
        "cls_w2": np.asarray(inputs["cls_w2"], f32),
        "cls_b2": np.asarray(inputs["cls_b2"], f32).reshape(1, 2),
    }
    in_maps = []
    for i in range(NCORES):
        xs = x[i * NS : (i + 1) * NS]
        xs = np.pad(xs, ((0, NSP - NS), (0, 0)))
        m = dict(base)
        m["x"] = np.ascontiguousarray(xs)
        in_maps.append(m)

    res = run_bass_kernel_spmd(_NC, in_maps, core_ids=list(range(NCORES)))
    pred = np.concatenate([res.results[i]["pred"][:NS] for i in range(NCORES)], axis=0)
    z = np.concatenate([res.results[i]["z"][:NS] for i in range(NCORES)], axis=0)
    alog = res.results[0]["alog"]
    anchor_vec = np.asarray(inputs["anchors"], f32)[None]
    return pred, z, anchor_vec, alog


# revision 24
# speedup vs baseline: 1.0896x; 1.0896x over previous
"""AnchorGNN forward on 8 TRN2 NeuronCores (Bass/Tile), nodes sharded 8-way.

Self-contained: hardcodes all shapes from the problem spec.
kernel(**inputs) takes the full unsharded inputs and returns
(pred [N,2], z [N,128], anchor_vec [1,A,128], anchor_logits [A,2]).

Layout strategy: activations live feature-major ([128 feat partitions, nodes
free]) so every matmul's stationary operand is a natural weight tile and
per-feature biases are per-partition scalars. The anchor-graph learner (the
dominant cost) runs the relu as a dual-op tensor_scalar on DVE (bf16, 4x
mode) and contracts on the PE with per-anchor column-masked stationary tiles
so all 32 anchors accumulate into one [32, chunk] PSUM tile. The node-dim
reductions (colsum, adj.T @ support) are computed as local partials and
combined with one tiny [32,129] AllReduce per hop; node_norm/anchor_norm are
never materialized (their scalings are factored onto the tiny side of each
product).
"""
import numpy as np

import concourse.bass as bass
import concourse.mybir as mybir
import concourse.tile as tile
from concourse.bass_utils import run_bass_kernel_spmd
from concourse.masks import make_identity

F32 = mybir.dt.float32
BF16 = mybir.dt.bfloat16
AF = mybir.ActivationFunctionType
ALU = mybir.AluOpType

N, NFEAT, NHID, A, P, HOPS = 50000, 256, 128, 32, 4, 2
NCORES = 8
NS = N // NCORES         # 6250 nodes per core
NSP = 6272               # padded to 49*128
NCH = NSP // 128         # 49 chunks of 128 nodes
EPS = 1e-12
LN_EPS = 1e-5

# matmul free-dim chunking of NSP (PSUM bank = 512 f32)
MM = []
_off = 0
for _w in [512] * 12 + [128]:
    MM.append((_off, _w))
    _off += _w
# learner super-chunks (relu granularity)
SUPS = []
_off = 0
for _w in [2048] * 3 + [128]:
    SUPS.append((_off, _w))
    _off += _w

# anchor -> (col-group, mask position) for 3-way PE column tiling
GOFF = [0, 11, 22]          # first anchor of each group
GSIZ = [11, 11, 10]
PR = 96                     # permuted anchor rows (3 groups x 32)


def j_to_gm(j):
    g = 0 if j < 11 else (1 if j < 22 else 2)
    return g, j - GOFF[g]


_MAX_WAITS = 1


_SPLIT_ENGINES = {
    mybir.EngineType.PE, mybir.EngineType.Activation, mybir.EngineType.DVE,
    mybir.EngineType.Pool, mybir.EngineType.SP,
}


def _split_waits(nc):
    """This walrus build rejects >1 sync-wait per TPB instruction; hoist extras
    onto preceding same-engine NoOps (streams are in-order, so equivalent).
    DMA-queue instructions lower via DGE rings and are left untouched."""
    ctr = [0]

    def fresh_nop(engine, waits):
        ctr[0] += 1
        nop = mybir.InstNoOp(name=f"I-waitsplit-{ctr[0]}", ins=[], outs=[])
        nop.engine = engine
        nop.sync_info = mybir.SyncInfo(on_wait=list(waits), on_update=[])
        return nop

    for fn in nc.m.functions:
        for blk in fn.blocks:
            new_insts = []
            for inst in blk.instructions:
                si = getattr(inst, "sync_info", None)
                eng = getattr(inst, "engine", None)
                if (
                    si is not None
                    and eng in _SPLIT_ENGINES
                    and si.on_wait is not None
                    and len(si.on_wait) > _MAX_WAITS
                ):
                    waits = list(si.on_wait)
                    extra, keep = waits[:-_MAX_WAITS], waits[-_MAX_WAITS:]
                    for i in range(0, len(extra), _MAX_WAITS):
                        new_insts.append(fresh_nop(eng, extra[i : i + _MAX_WAITS]))
                    inst.sync_info = mybir.SyncInfo(
                        on_wait=keep, on_update=list(si.on_update or [])
                    )
                new_insts.append(inst)
            blk.instructions[:] = new_insts
    return nc


def build_nc(split_waits=True):
    nc = bass.Bass("TRN2", target_bir_lowering=False, debug=False, num_devices=NCORES)

    def inp(name, shape):
        return nc.dram_tensor(name, shape, F32, kind="ExternalInput").ap()

    x_d = inp("x", [NSP, NFEAT])
    im_w1_d = inp("im_w1", [NFEAT, NHID])
    im_b1_d = inp("im_b1", [1, NHID])
    im_w2_d = inp("im_w2", [NHID, NHID])
    im_b2_d = inp("im_b2", [1, NHID])
    anchors_d = inp("anchors", [A, NHID])
    ml_w1_d = inp("ml_w1", [P * 2 * NHID, NHID])   # host-reshaped
    ml_b1_d = inp("ml_b1", [P, NHID])
    ml_w2_d = inp("ml_w2", [P, NHID])
    ml_b2_d = inp("ml_b2", [1, P])
    enc_w_d = inp("enc_w", [HOPS * NHID, NHID])    # host-reshaped
    enc_b_d = inp("enc_b", [HOPS, NHID])
    ln_g_d = inp("ln_g", [1, NHID])
    ln_b_d = inp("ln_b", [1, NHID])
    cls_w1_d = inp("cls_w1", [NHID, NHID])
    cls_b1_d = inp("cls_b1", [1, NHID])
    prelu_a_d = inp("prelu_a", [1, NHID])
    cls_w2_d = inp("cls_w2", [NHID, 2])
    cls_b2_d = inp("cls_b2", [1, 2])

    z_d = nc.dram_tensor("z", [NSP, NHID], F32, kind="ExternalOutput").ap()
    pred_d = nc.dram_tensor("pred", [NSP, 2], F32, kind="ExternalOutput").ap()
    alog_d = nc.dram_tensor("alog", [A, 2], F32, kind="ExternalOutput").ap()

    with tile.TileContext(nc) as tc:
        with (
            tc.tile_pool(name="wts", bufs=1) as wp,
            tc.tile_pool(name="bigs", bufs=1) as bigp,
            tc.tile_pool(name="hxp", bufs=2) as hxp,
            tc.tile_pool(name="xin", bufs=3) as xp,
            tc.tile_pool(name="work", bufs=3) as wk,
            tc.tile_pool(name="tmpl", bufs=4) as tmpl,
            tc.tile_pool(name="psmm", bufs=3, space="PSUM") as psmm,
            tc.tile_pool(name="psat", bufs=1, space="PSUM") as psat,
            tc.tile_pool(name="dram", bufs=1, space="DRAM") as drp,
        ):
            # ============ weights & small derived tiles ============
            ident = wp.tile([128, 128], F32)
            make_identity(nc, ident)
            identb = wp.tile([128, 128], BF16)
            nc.vector.tensor_copy(identb[:, :], ident[:, :])
            ones_r = wp.tile([1, 128], F32)
            nc.vector.memset(ones_r[:, :], 1.0)
            ones_rb = wp.tile([1, 128], BF16)
            nc.vector.memset(ones_rb[:, :], 1.0)

            def load_f32(name, src_ap, p, f):
                t = wp.tile([p, f], F32, name=name)
                nc.sync.dma_start(out=t[:, :], in_=src_ap)
                return t

            def cast_bf16(name, src, p, f):
                t = wp.tile([p, f], BF16, name=name)
                nc.vector.tensor_copy(t[:, :], src[:, :])
                return t

            def load_bf16(name, src_ap, p, f):
                return cast_bf16(name, load_f32(name + "_f", src_ap, p, f), p, f)

            im_w1b = [
                load_bf16(f"im_w1b{k}", im_w1_d[k * 128 : (k + 1) * 128, :], 128, NHID)
                for k in range(2)
            ]
            im_w2b = load_bf16("im_w2b", im_w2_d[:, :], 128, NHID)
            Wxb = [
                load_bf16(f"Wxb{p}", ml_w1_d[p * 256 : p * 256 + 128, :], 128, NHID)
                for p in range(P)
            ]
            Wab = [
                load_bf16(f"Wab{p}", ml_w1_d[p * 256 + 128 : p * 256 + 256, :], 128, NHID)
                for p in range(P)
            ]
            enc_wb = [
                load_bf16(f"enc_wb{i}", enc_w_d[i * 128 : (i + 1) * 128, :], 128, NHID)
                for i in range(HOPS)
            ]
            cls_w1b = load_bf16("cls_w1b", cls_w1_d[:, :], 128, NHID)
            cls_w2b = load_bf16("cls_w2b", cls_w2_d[:, :], 128, 2)

            # column-vector biases via outer product with ones ([1,1] rhs)
            def bias_col(name, src_ap, length):
                row = load_f32(name + "_r", src_ap, 1, length)
                ps = psmm.tile([length, 1], F32, name=name + "_ps", tag="mm")
                nc.tensor.matmul(ps[:, :], row[:, :], ones_r[0:1, 0:1],
                                 start=True, stop=True)
                col = wp.tile([length, 1], F32, name=name)
                nc.scalar.copy(col[:, :], ps[:, :])
                return col

            im_b1c = bias_col("im_b1c", im_b1_d[:, :], NHID)
            im_b2c = bias_col("im_b2c", im_b2_d[:, :], NHID)
            enc_bc = [
                bias_col(f"enc_bc{i}", enc_b_d[i : i + 1, :], NHID) for i in range(HOPS)
            ]
            cls_b1c = bias_col("cls_b1c", cls_b1_d[:, :], NHID)
            prelu_ac = bias_col("prelu_ac", prelu_a_d[:, :], NHID)
            cls_b2c = bias_col("cls_b2c", cls_b2_d[:, :], 2)
            neg_b1c = wp.tile([NHID, 1], F32, name="neg_b1c")
            nc.vector.tensor_scalar(out=neg_b1c[:, :], in0=cls_b1c[:, :],
                                    scalar1=-1.0, scalar2=None, op0=ALU.mult)

            # LN gamma/beta broadcast tiles [128 nodes, 128 feat]
            def bcast_row(name, src_ap):
                row = load_f32(name + "_r", src_ap, 1, NHID)
                ps = psmm.tile([128, NHID], F32, name=name + "_ps", tag="mm")
                nc.tensor.matmul(ps[:, :], ones_r[:, :], row[:, :],
                                 start=True, stop=True)
                t = wp.tile([128, NHID], BF16, name=name)
                nc.scalar.copy(t[:, :], ps[:, :])
                return t

            gBb = bcast_row("gBb", ln_g_d[:, :])
            bBb = bcast_row("bBb", ln_b_d[:, :])

            # anchorsT [128, 32] bf16
            anchors_f = load_f32("anchors_f", anchors_d[:, :], A, NHID)
            anchors_fb = cast_bf16("anchors_fb", anchors_f, A, NHID)
            aT_ps = psmm.tile([NHID, A], BF16, name="aT_ps", tag="mm")
            nc.tensor.transpose(aT_ps[:, :], anchors_fb[:, :], identb[0:A, 0:A])
            anchorsTb = wp.tile([NHID, A], BF16, name="anchorsTb")
            nc.scalar.copy(anchorsTb[:, :], aT_ps[:, :])

            # ml_b1T [128, P] f32 ; ml_w2T [128, P]
            ml_b1_f = load_f32("ml_b1_f", ml_b1_d[:, :], P, NHID)
            mb1_ps = psmm.tile([NHID, P], F32, name="mb1_ps", tag="mm")
            nc.tensor.transpose(mb1_ps[:, :], ml_b1_f[:, :], ident[0:P, 0:P])
            ml_b1T = wp.tile([NHID, P], F32, name="ml_b1T")
            nc.scalar.copy(ml_b1T[:, :], mb1_ps[:, :])

            ml_w2_f = load_f32("ml_w2_f", ml_w2_d[:, :], P, NHID)
            mw2_ps = psmm.tile([NHID, P], F32, name="mw2_ps", tag="mm")
            nc.tensor.transpose(mw2_ps[:, :], ml_w2_f[:, :], ident[0:P, 0:P])
            ml_w2Tb = wp.tile([NHID, P], BF16, name="ml_w2Tb")
            nc.scalar.copy(ml_w2Tb[:, :], mw2_ps[:, :])

            # masked stationary tiles: per (p, mask-pos m), [128, 32] with
            # w2_p in column m; shared by the 3 column-group tiles.
            w2m = wp.tile([128, P * 11 * 32], BF16, name="w2m")
            nc.vector.memset(w2m[:, :], 0.0)
            for p in range(P):
                for m in range(11):
                    cidx = (p * 11 + m) * 32 + m
                    nc.vector.tensor_copy(
                        w2m[:, cidx : cidx + 1], ml_w2Tb[:, p : p + 1]
                    )

            # b2 per-head bias columns on 32 partitions: [32, P]
            mlb2_row = load_f32("mlb2_row", ml_b2_d[:, :], 1, P)
            b2_ps = psmm.tile([128, P], F32, name="b2_ps", tag="mm")
            nc.tensor.matmul(b2_ps[:, :], ones_r[0:1, :], mlb2_row[:, :],
                             start=True, stop=True)
            b2col = wp.tile([128, P], F32, name="b2col")
            nc.scalar.copy(b2col[:, :], b2_ps[:, :])

            # perm-row validity mask [PR, 1] (1.0 on rows holding an anchor)
            selr = wp.tile([1, PR], F32, name="selr")
            nc.vector.memset(selr[:, :], 0.0)
            for g in range(3):
                nc.vector.memset(selr[0:1, 32 * g : 32 * g + GSIZ[g]], 1.0)
            selrb = wp.tile([1, PR], BF16, name="selrb")
            nc.vector.tensor_copy(selrb[:, :], selr[:, :])
            mc_ps = psmm.tile([PR, 1], F32, name="mc_ps", tag="mm")
            nc.tensor.matmul(mc_ps[:, :], selr[:, :], ones_r[0:1, 0:1],
                             start=True, stop=True)
            mask_col = wp.tile([PR, 1], F32, name="mask_col")
            nc.scalar.copy(mask_col[:, :], mc_ps[:, :])
            mrb_ps = psmm.tile([128, PR], F32, name="mrb_ps", tag="mm")
            nc.tensor.matmul(mrb_ps[:, :], ones_rb[:, :], selrb[:, :],
                             start=True, stop=True)
            maskRowB = wp.tile([128, PR], BF16, name="maskRowB")
            nc.scalar.copy(maskRowB[:, :], mrb_ps[:, :])

            # ============ input mapper: x -> hT [128, NSP] bf16 ============
            hT = bigp.tile([NHID, NSP], BF16, name="hT")
            for (off, w) in MM:
                nsub = w // 128
                xas = []
                for sNum in range(nsub):
                    c = off // 128 + sNum
                    xa = xp.tile([128, NFEAT], F32, name=f"xa{c}", tag=f"xa{sNum}")
                    nc.sync.dma_start(out=xa[:, :],
                                      in_=x_d[c * 128 : (c + 1) * 128, :])
                    xab = xp.tile([128, NFEAT], BF16, name=f"xab{c}", tag=f"xab{sNum}")
                    nc.vector.tensor_copy(xab[:, :], xa[:, :])
                    xas.append(xab)
                # transposed x: [256 feat(2 ktiles), w nodes]
                xt_ps = [
                    psmm.tile([128, w], BF16, name=f"xt_ps{off}_{k}", tag="mm")
                    for k in range(2)
                ]
                for k in range(2):
                    for sNum in range(nsub):
                        nc.tensor.transpose(
                            xt_ps[k][:, sNum * 128 : (sNum + 1) * 128],
                            xas[sNum][:, k * 128 : (k + 1) * 128],
                            identb[:, :],
                        )
                xtb = wk.tile([128, 2 * w], BF16, name=f"xtb{off}", tag="xtb")
                nc.vector.tensor_copy(xtb[:, 0:w], xt_ps[0][:, :])
                nc.vector.tensor_copy(xtb[:, w : 2 * w], xt_ps[1][:, :])
                h1_ps = psmm.tile([NHID, w], F32, name=f"h1_ps{off}", tag="mm")
                for k in range(2):
                    nc.tensor.matmul(
                        h1_ps[:, :], im_w1b[k][:, :], xtb[:, k * w : (k + 1) * w],
                        start=(k == 0), stop=(k == 1),
                    )
                r1 = wk.tile([NHID, w], BF16, name=f"r1{off}", tag="r1")
                nc.vector.tensor_scalar(
                    out=r1[:, :], in0=h1_ps[:, :], scalar1=im_b1c[:, :],
                    scalar2=0.0, op0=ALU.add, op1=ALU.max,
                )
                h2_ps = psmm.tile([NHID, w], F32, name=f"h2_ps{off}", tag="mm")
                nc.tensor.matmul(h2_ps[:, :], im_w2b[:, :], r1[:, :],
                                 start=True, stop=True)
                nc.vector.tensor_scalar(
                    out=hT[:, off : off + w], in0=h2_ps[:, :],
                    scalar1=im_b2c[:, :], scalar2=None, op0=ALU.add,
                )

            # hop-0 support depends only on hT; emit early so its PE/ACT work
            # overlaps the learner instead of the serial tail.
            def support_nm_fwd(src_T, w_b, name, s_nm=None, mm_list=None):
                if s_nm is None:
                    s_nm = bigp.tile([128, NSP], BF16, name=name, tag="s_nm")
                for (off, w) in (mm_list if mm_list is not None else MM):
                    sp_ps = psmm.tile([NHID, w], F32, name=f"{name}_ps{off}", tag="mm")
                    nc.tensor.matmul(sp_ps[:, :], w_b[:, :], src_T[:, off : off + w],
                                     start=True, stop=True)
                    sT = wk.tile([NHID, w], BF16, name=f"{name}_sT{off}", tag="sT")
                    nc.scalar.copy(sT[:, :], sp_ps[:, :])
                    tr_ps = psmm.tile([128, w], BF16, name=f"{name}_tr{off}", tag="mm")
                    nsub = w // 128
                    for sNum in range(nsub):
                        nc.tensor.transpose(
                            tr_ps[:, sNum * 128 : (sNum + 1) * 128],
                            sT[:, sNum * 128 : (sNum + 1) * 128],
                            identb[:, :],
                        )
                    nc.scalar.copy(s_nm[:, off : off + w], tr_ps[:, :])
                return s_nm

            s0_nm = bigp.tile([128, NSP], BF16, name="s0_nm", tag="s_nm")
            S0_CHUNKS = [MM[0:3], MM[3:6], MM[6:9], MM[9:13]]

            # ============ learner: adj_perm [PR, NSP] bf16 ============
            # (row 32g+m holds anchor GOFF[g]+m; other rows are sigmoid(b2))
            adjT = bigp.tile([PR, NSP], BF16, name="adjT")
            for p in range(P):
                support_nm_fwd(hT, enc_wb[0], f"s0nm{p}", s_nm=s0_nm,
                               mm_list=S0_CHUNKS[p])
                # c_pT = anchors @ Wa_p (transposed) + ml_b1[p]
                ha_ps = psmm.tile([NHID, A], F32, name=f"ha_ps{p}", tag="mm")
                nc.tensor.matmul(ha_ps[:, :], Wab[p][:, :], anchorsTb[:, :],
                                 start=True, stop=True)
                c_pT = wk.tile([NHID, A], F32, name=f"c_pT{p}", tag="cpT")
                nc.vector.tensor_scalar(
                    out=c_pT[:, :], in0=ha_ps[:, :],
                    scalar1=ml_b1T[:, p : p + 1], scalar2=None, op0=ALU.add,
                )
                # hx_p = h @ Wx_p (transposed), bf16
                hxb = hxp.tile([NHID, NSP], BF16, name=f"hxb{p}", tag="hxb")
                for (off, w) in MM:
                    hx_ps = psmm.tile([NHID, w], F32, name=f"hx_ps{p}_{off}", tag="mm")
                    nc.tensor.matmul(hx_ps[:, :], Wxb[p][:, :], hT[:, off : off + w],
                                     start=True, stop=True)
                    nc.scalar.copy(hxb[:, off : off + w], hx_ps[:, :])
                # per super-chunk: relu per anchor, masked matmuls, sigmoid
                for (soff, sw) in SUPS:
                    nbank = (sw + 511) // 512
                    pss = [
                        psat.tile([PR, min(512, sw - k * 512)], F32,
                                  name=f"at{p}_{soff}_{k}", tag=f"at{k}")
                        for k in range(nbank)
                    ]
                    jseq = []
                    for m in range(11):
                        for g in range(3):
                            if m < GSIZ[g]:
                                jseq.append((g, m, GOFF[g] + m))
                    for (g, m, j) in jseq:
                        t = tmpl.tile([128, sw], BF16, name=f"t{p}{soff}{j}", tag="t")
                        if j % 4 == 3:
                            nc.scalar.activation(
                                t[:, :], hxb[:, soff : soff + sw], AF.Relu,
                                bias=c_pT[:, j : j + 1],
                            )
                        else:
                            nc.vector.tensor_scalar(
                                out=t[:, :], in0=hxb[:, soff : soff + sw],
                                scalar1=c_pT[:, j : j + 1], scalar2=0.0,
                                op0=ALU.add, op1=ALU.max,
                            )
                        mcol = (p * 11 + m) * 32
                        for k in range(nbank):
                            kw = min(512, sw - k * 512)
                            nc.tensor.matmul(
                                pss[k][32 * g : 32 * (g + 1), :],
                                w2m[:, mcol : mcol + 32],
                                t[:, k * 512 : k * 512 + kw],
                                start=(m == 0), stop=(m == GSIZ[g] - 1),
                                tile_position=(0, 32 * g),
                                skip_group_check=True,
                            )
                    for k in range(nbank):
                        kw = min(512, sw - k * 512)
                        dst = adjT[:, soff + k * 512 : soff + k * 512 + kw]
                        if p == 0:
                            nc.scalar.activation(
                                dst, pss[k][:, :], AF.Sigmoid,
                                bias=b2col[0:PR, p : p + 1],
                            )
                        else:
                            sg = wk.tile([PR, kw], BF16, name=f"sg{p}{soff}{k}", tag="sg")
                            nc.scalar.activation(
                                sg[:, :], pss[k][:, :], AF.Sigmoid,
                                bias=b2col[0:PR, p : p + 1],
                            )
                            nc.vector.tensor_tensor(
                                out=dst, in0=dst, in1=sg[:, :], op=ALU.add
                            )

            # zero the padded node columns so node-reductions are exact
            nc.vector.memset(adjT[:, NS:NSP], 0.0)

            # ============ norms prep ============
            colsum = bigp.tile([PR, 1], F32, name="colsum")
            nc.vector.tensor_reduce(colsum[:, :], adjT[:, :],
                                    mybir.AxisListType.X, ALU.add)

            # adj_nm [128, NCH*32] bf16 : node-major chunks of adjT
            adj_nm = hxp.tile([128, NCH * PR], BF16, name="adj_nm", tag="hxb")
            for (off, w) in MM:
                nsub = w // 128
                tr_ps = psmm.tile([128, nsub * PR], BF16,
                                  name=f"adjnm_ps{off}", tag="mm")
                for s in range(nsub):
                    c = off // 128 + s
                    nc.tensor.transpose(
                        tr_ps[:, s * PR : (s + 1) * PR],
                        adjT[:, c * 128 : (c + 1) * 128],
                        identb[0:PR, 0:PR],
                    )
                nc.scalar.copy(
                    adj_nm[:, (off // 128) * PR : (off // 128 + nsub) * PR],
                    tr_ps[:, :],
                )

            # rowsum per node -> rinv broadcast tile [128, NSP] bf16
            rs_nm = bigp.tile([128, NCH], F32, name="rs_nm")
            for c in range(NCH):
                amk = wk.tile([128, PR], BF16, name=f"amk{c}", tag="amk")
                nc.vector.tensor_tensor(
                    out=amk[:, :], in0=adj_nm[:, c * PR : (c + 1) * PR],
                    in1=maskRowB[:, :], op=ALU.mult,
                )
                nc.vector.tensor_reduce(
                    rs_nm[:, c : c + 1], amk[:, :],
                    mybir.AxisListType.X, ALU.add,
                )
            nc.vector.tensor_scalar(out=rs_nm[:, :], in0=rs_nm[:, :],
                                    scalar1=EPS, scalar2=None, op0=ALU.max)
            rinv_nm = bigp.tile([128, NCH], F32, name="rinv_nm")
            nc.vector.reciprocal(rinv_nm[:, :], rs_nm[:, :])
            # rearrange to free-major [1, NSP] via per-column PE transposes
            rinv_fm = bigp.tile([1, NSP], BF16, name="rinv_fm")
            for (off, w) in MM:
                nsub = w // 128
                rt_ps = psmm.tile([1, w], F32, name=f"rt_ps{off}", tag="mm")
                for s in range(nsub):
                    c = off // 128 + s
                    nc.tensor.transpose(
                        rt_ps[:, s * 128 : (s + 1) * 128],
                        rinv_nm[:, c : c + 1], ident[:, :],
                    )
                nc.scalar.copy(rinv_fm[0:1, off : off + w], rt_ps[:, :])
            rinvB = bigp.tile([128, NSP], BF16, name="rinvB")
            for (off, w) in MM:
                nsub = w // 128
                rb_ps = psmm.tile([128, w], F32, name=f"rb_ps{off}", tag="mm")
                for s in range(nsub):
                    nc.tensor.matmul(
                        rb_ps[:, s * 128 : (s + 1) * 128],
                        ones_rb[:, :],
                        rinv_fm[0:1, off + s * 128 : off + (s + 1) * 128],
                        start=True, stop=True,
                    )
                nc.scalar.copy(rinvB[:, off : off + w], rb_ps[:, :])

            # ============ GNN hops ============
            support_nm = support_nm_fwd

            def agg_raw(s_nm, name):
                """[PR, 128] PSUM partial of adj.T @ support (over local nodes)."""
                ps = psat.tile([PR, NHID], F32, name=name, tag="at0")
                for c in range(NCH):
                    nc.tensor.matmul(
                        ps[:, :], adj_nm[:, c * PR : (c + 1) * PR],
                        s_nm[:, c * 128 : (c + 1) * 128],
                        start=(c == 0), stop=(c == NCH - 1),
                    )
                return ps

            def hop_apply(agg_b, enc_bc_i, name):
                """xc_next.T = relu((agg.T @ adjT) * rinvB + enc_b)."""
                xcT = bigp.tile([NHID, NSP], BF16, name=name)
                for (off, w) in MM:
                    xp_ps = psmm.tile([NHID, w], F32, name=f"{name}_ps{off}", tag="mm")
                    nc.tensor.matmul(xp_ps[:, :], agg_b[:, :], adjT[:, off : off + w],
                                     start=True, stop=True)
                    xpre = wk.tile([NHID, w], BF16, name=f"{name}_pre{off}", tag="sT")
                    nc.scalar.copy(xpre[:, :], xp_ps[:, :])
                    xm = wk.tile([NHID, w], BF16, name=f"{name}_m{off}", tag="xm")
                    nc.vector.tensor_tensor(
                        out=xm[:, :], in0=xpre[:, :],
                        in1=rinvB[:, off : off + w], op=ALU.mult,
                    )
                    nc.vector.tensor_scalar(
                        out=xcT[:, off : off + w], in0=xm[:, :],
                        scalar1=enc_bc_i[:, :], scalar2=0.0,
                        op0=ALU.add, op1=ALU.max,
                    )
                return xcT

            # hop 0: fused AllReduce of [agg0_raw | colsum]
            agg0_ps = agg_raw(s0_nm, "agg0_ps")
            arbuf = bigp.tile([PR, 132], F32, name="arbuf")
            nc.scalar.copy(arbuf[:, 0:NHID], agg0_ps[:, :])
            nc.vector.tensor_copy(arbuf[:, NHID : NHID + 1], colsum[:, :])
            drin0 = drp.tile([PR, 129], F32, name="drin0")
            drout0 = drp.tile([PR, 129], F32, name="drout0")
            nc.sync.dma_start(out=drin0[:, :], in_=arbuf[:, 0:129])
            nc.gpsimd.collective_compute(
                "AllReduce", ALU.add,
                replica_groups=[list(range(NCORES))],
                ins=[drin0[:, :].opt()], outs=[drout0[:, :].opt()],
            )
            arres0 = bigp.tile([PR, 132], F32, name="arres0")
            nc.sync.dma_start(out=arres0[:, 0:129], in_=drout0[:, :])

            csc = bigp.tile([PR, 1], F32, name="csc")
            nc.vector.tensor_scalar(out=csc[:, :], in0=arres0[:, NHID : NHID + 1],
                                    scalar1=EPS, scalar2=None, op0=ALU.max)
            csinv = bigp.tile([PR, 1], F32, name="csinv")
            nc.vector.reciprocal(csinv[:, :], csc[:, :])
            # fold the perm-row validity mask into the scale
            nc.vector.tensor_scalar(out=csinv[:, :], in0=csinv[:, :],
                                    scalar1=mask_col[:, :], scalar2=None,
                                    op0=ALU.mult)
            agg0b = bigp.tile([PR, NHID], BF16, name="agg0b")
            nc.vector.tensor_scalar(out=agg0b[:, :], in0=arres0[:, 0:NHID],
                                    scalar1=csinv[:, :], scalar2=None, op0=ALU.mult)

            xc1T = hop_apply(agg0b, enc_bc[0], "xc1T")

            # hop 1
            s1_nm = support_nm(xc1T, enc_wb[1], "s1_nm")
            agg1_ps = agg_raw(s1_nm, "agg1_ps")
            arbuf1 = bigp.tile([PR, NHID], F32, name="arbuf1")
            nc.scalar.copy(arbuf1[:, :], agg1_ps[:, :])
            drin1 = drp.tile([PR, NHID], F32, name="drin1")
            drout1 = drp.tile([PR, NHID], F32, name="drout1")
            nc.sync.dma_start(out=drin1[:, :], in_=arbuf1[:, :])
            nc.gpsimd.collective_compute(
                "AllReduce", ALU.add,
                replica_groups=[list(range(NCORES))],
                ins=[drin1[:, :].opt()], outs=[drout1[:, :].opt()],
            )
            arres1 = bigp.tile([PR, NHID], F32, name="arres1")
            nc.sync.dma_start(out=arres1[:, :], in_=drout1[:, :])
            agg1b = bigp.tile([PR, NHID], BF16, name="agg1b")
            nc.vector.tensor_scalar(out=agg1b[:, :], in0=arres1[:, :],
                                    scalar1=csinv[:, :], scalar2=None, op0=ALU.mult)

            xc2T = hop_apply(agg1b, enc_bc[1], "xc2T")

            # ============ residual + LN (node-major) ============
            zTb = bigp.tile([NHID, NSP], BF16, name="zTb", tag="xc1T")
            for (off, w) in MM:
                nc.vector.tensor_tensor(
                    out=zTb[:, off : off + w], in0=xc2T[:, off : off + w],
                    in1=hT[:, off : off + w], op=ALU.add,
                )
            z_nm = bigp.tile([128, NSP], BF16, name="z_nm", tag="s_nm")
            for (off, w) in MM:
                tr_ps = psmm.tile([128, w], BF16, name=f"znm_ps{off}", tag="mm")
                nsub = w // 128
                for s in range(nsub):
                    nc.tensor.transpose(
                        tr_ps[:, s * 128 : (s + 1) * 128],
                        zTb[:, off + s * 128 : off + (s + 1) * 128],
                        identb[:, :],
                    )
                nc.scalar.copy(z_nm[:, off : off + w], tr_ps[:, :])

            mu_all = bigp.tile([128, NCH], F32, name="mu_all")
            sq_all = bigp.tile([128, NCH], F32, name="sq_all")
            for (off, w) in MM:
                nsub = w // 128
                c0 = off // 128
                zc3 = z_nm[:, off : off + w].rearrange("p (c f) -> p c f", f=128)
                nc.vector.tensor_reduce(mu_all[:, c0 : c0 + nsub], zc3,
                                        mybir.AxisListType.X, ALU.add)
                sqt = wk.tile([128, w], BF16, name=f"sqt{off}", tag="sqt", bufs=1)
                nc.vector.tensor_tensor(out=sqt[:, :], in0=z_nm[:, off : off + w],
                                        in1=z_nm[:, off : off + w], op=ALU.mult)
                nc.vector.tensor_reduce(
                    sq_all[:, c0 : c0 + nsub],
                    sqt[:, :].rearrange("p (c f) -> p c f", f=128),
                    mybir.AxisListType.X, ALU.add)
            nc.vector.tensor_scalar(out=mu_all[:, :], in0=mu_all[:, :],
                                    scalar1=1.0 / NHID, scalar2=None, op0=ALU.mult)
            nc.vector.tensor_scalar(out=sq_all[:, :], in0=sq_all[:, :],
                                    scalar1=1.0 / NHID, scalar2=None, op0=ALU.mult)
            msq = bigp.tile([128, NCH], F32, name="msq")
            nc.vector.tensor_tensor(out=msq[:, :], in0=mu_all[:, :],
                                    in1=mu_all[:, :], op=ALU.mult)
            var = bigp.tile([128, NCH], F32, name="var")
            nc.vector.tensor_tensor(out=var[:, :], in0=sq_all[:, :],
                                    in1=msq[:, :], op=ALU.subtract)
            stdt = bigp.tile([128, NCH], F32, name="stdt")
            lneps_c = wp.tile([128, 1], F32, name="lneps_c")
            nc.vector.memset(lneps_c[:, :], LN_EPS)
            nc.scalar.activation(stdt[:, :], var[:, :], AF.Sqrt, bias=lneps_c[:, :])
            rstd = bigp.tile([128, NCH], F32, name="rstd")
            nc.vector.reciprocal(rstd[:, :], stdt[:, :])

            znT = bigp.tile([NHID, NSP], BF16, name="znT", tag="rinvB")
            for (off, w) in MM:
                nsub = w // 128
                trz_ps = psmm.tile([128, w], F32, name=f"znt_ps{off}", tag="mm")
                for s in range(nsub):
                    c = off // 128 + s
                    zc = z_nm[:, c * 128 : (c + 1) * 128]
                    t1 = wk.tile([128, 128], BF16, name=f"lnt1_{c}", tag="lnt1")
                    nc.vector.tensor_scalar(
                        out=t1[:, :], in0=zc,
                        scalar1=mu_all[:, c : c + 1], scalar2=rstd[:, c : c + 1],
                        op0=ALU.subtract, op1=ALU.mult,
                    )
                    t2 = wk.tile([128, 128], BF16, name=f"lnt2_{c}", tag="lnt2")
                    nc.vector.tensor_tensor(out=t2[:, :], in0=t1[:, :],
                                            in1=gBb[:, :], op=ALU.mult)
                    znf = wk.tile([128, 128], F32, name=f"znf_{c}", tag="znf")
                    nc.vector.tensor_tensor(out=znf[:, :], in0=t2[:, :],
                                            in1=bBb[:, :], op=ALU.add)
                    if c * 128 < NS:
                        hi = min(128, NS - c * 128)
                        nc.sync.dma_start(
                            out=z_d[c * 128 : c * 128 + hi, :], in_=znf[0:hi, :]
                        )
                    nc.tensor.transpose(
                        trz_ps[:, s * 128 : (s + 1) * 128], znf[:, :], ident[:, :]
                    )
                if True:
                    nc.scalar.copy(znT[:, off : off + w], trz_ps[:, :])

            # ============ classifier ============
            for (off, w) in MM:
                t1_ps = psmm.tile([NHID, w], F32, name=f"cls_ps{off}", tag="mm")
                nc.tensor.matmul(t1_ps[:, :], cls_w1b[:, :], znT[:, off : off + w],
                                 start=True, stop=True)
                r1p = wk.tile([NHID, w], BF16, name=f"r1p{off}", tag="sT")
                nc.scalar.activation(r1p[:, :], t1_ps[:, :], AF.Relu,
                                     bias=cls_b1c[:, :])
                r2p = wk.tile([NHID, w], BF16, name=f"r2p{off}", tag="xm")
                nc.scalar.activation(r2p[:, :], t1_ps[:, :], AF.Relu,
                                     bias=neg_b1c[:, :], scale=-1.0)
                r2s = wk.tile([NHID, w], BF16, name=f"r2s{off}", tag="r2s")
                nc.vector.tensor_scalar(out=r2s[:, :], in0=r2p[:, :],
                                        scalar1=prelu_ac[:, :], scalar2=None,
                                        op0=ALU.mult)
                t1b = wk.tile([NHID, w], BF16, name=f"t1b{off}", tag="t1b")
                nc.vector.tensor_tensor(out=t1b[:, :], in0=r1p[:, :],
                                        in1=r2s[:, :], op=ALU.subtract)
                pr_ps = psmm.tile([2, w], F32, name=f"pr_ps{off}", tag="mm")
                nc.tensor.matmul(pr_ps[:, :], cls_w2b[:, :], t1b[:, :],
                                 start=True, stop=True)
                predc = wk.tile([2, w], F32, name=f"predc{off}", tag="predc")
                nc.scalar.activation(predc[:, :], pr_ps[:, :], AF.Identity,
                                     bias=cls_b2c[:, :])
                nsub = w // 128
                pt_ps = psmm.tile([128, nsub * 2], F32, name=f"pt_ps{off}", tag="mm")
                for s in range(nsub):
                    nc.tensor.transpose(
                        pt_ps[:, s * 2 : (s + 1) * 2],
                        predc[:, s * 128 : (s + 1) * 128],
                        ident[0:2, 0:2],
                    )
                predn = wk.tile([128, nsub * 2], F32, name=f"predn{off}", tag="predn")
                nc.scalar.copy(predn[:, :], pt_ps[:, :])
                for s in range(nsub):
                    c = off // 128 + s
                    if c * 128 < NS:
                        hi = min(128, NS - c * 128)
                        nc.sync.dma_start(
                            out=pred_d[c * 128 : c * 128 + hi, :],
                            in_=predn[0:hi, s * 2 : (s + 1) * 2],
                        )

            # anchors through the classifier
            a1_ps = psmm.tile([NHID, A], F32, name="a1_ps", tag="mm")
            nc.tensor.matmul(a1_ps[:, :], cls_w1b[:, :], anchorsTb[:, :],
                             start=True, stop=True)
            ar1 = wk.tile([NHID, A], BF16, name="ar1", tag="cpT")
            nc.scalar.activation(ar1[:, :], a1_ps[:, :], AF.Relu,
                                 bias=cls_b1c[:, :])
            ar2 = wk.tile([NHID, A], BF16, name="ar2", tag="ar2")
            nc.scalar.activation(ar2[:, :], a1_ps[:, :], AF.Relu,
                                 bias=neg_b1c[:, :], scale=-1.0)
            ar2s = wk.tile([NHID, A], BF16, name="ar2s", tag="ar2s")
            nc.vector.tensor_scalar(out=ar2s[:, :], in0=ar2[:, :],
                                    scalar1=prelu_ac[:, :], scalar2=None,
                                    op0=ALU.mult)
            a1b = wk.tile([NHID, A], BF16, name="a1b", tag="a1b")
            nc.vector.tensor_tensor(out=a1b[:, :], in0=ar1[:, :],
                                    in1=ar2s[:, :], op=ALU.subtract)
            a2_ps = psmm.tile([2, A], F32, name="a2_ps", tag="mm")
            nc.tensor.matmul(a2_ps[:, :], cls_w2b[:, :], a1b[:, :],
                             start=True, stop=True)
            alsb = wk.tile([2, A], F32, name="alsb", tag="predc")
            nc.scalar.activation(alsb[:, :], a2_ps[:, :], AF.Identity,
                                 bias=cls_b2c[:, :])
            alt_ps = psmm.tile([A, 2], F32, name="alt_ps", tag="mm")
            nc.tensor.transpose(alt_ps[:, :], alsb[:, :], ident[0:2, 0:2])
            alog_sb = wk.tile([A, 2], F32, name="alog_sb", tag="alog")
            nc.scalar.copy(alog_sb[:, :], alt_ps[:, :])
            nc.sync.dma_start(out=alog_d[:, :], in_=alog_sb[:, :])

    if split_waits:
        _split_waits(nc)
    return nc


_NC = None


def kernel(**inputs):
    global _NC
    if _NC is None:
        _NC = build_nc()

    f32 = np.float32
    x = np.asarray(inputs["x"], f32)
    base = {
        "im_w1": np.asarray(inputs["im_w1"], f32),
        "im_b1": np.asarray(inputs["im_b1"], f32).reshape(1, NHID),
        "im_w2": np.asarray(inputs["im_w2"], f32),
        "im_b2": np.asarray(inputs["im_b2"], f32).reshape(1, NHID),
        "anchors": np.asarray(inputs["anchors"], f32),
        "ml_w1": np.asarray(inputs["ml_w1"], f32).reshape(P * 2 * NHID, NHID),
        "ml_b1": np.asarray(inputs["ml_b1"], f32),
        "ml_w2": np.asarray(inputs["ml_w2"], f32),
        "ml_b2": np.asarray(inputs["ml_b2"], f32).reshape(1, P),
        "enc_w": np.asarray(inputs["enc_w"], f32).reshape(HOPS * NHID, NHID),
        "enc_b": np.asarray(inputs["enc_b"], f32),
        "ln_g": np.asarray(inputs["ln_g"], f32).reshape(1, NHID),
        "ln_b": np.asarray(inputs["ln_b"], f32).reshape(1, NHID),
        "cls_w1": np.asarray(inputs["cls_w1"], f32),
        "cls_b1": np.asarray(inputs["cls_b1"], f32).reshape(1, NHID),
        "prelu_a": np.asarray(inputs["prelu_a"], f32).reshape(1, NHID),
        "cls_w2": np.asarray(inputs["cls_w2"], f32),
        "cls_b2": np.asarray(inputs["cls_b2"], f32).reshape(1, 2),
    }
    in_maps = []
    for i in range(NCORES):
        xs = x[i * NS : (i + 1) * NS]
        xs = np.pad(xs, ((0, NSP - NS), (0, 0)))
        m = dict(base)
        m["x"] = np.ascontiguousarray(xs)
        in_maps.append(m)

    res = run_bass_kernel_spmd(_NC, in_maps, core_ids=list(range(NCORES)))
    pred = np.concatenate([res.results[i]["pred"][:NS] for i in range(NCORES)], axis=0)
    z = np.concatenate([res.results[i]["z"][:NS] for i in range(NCORES)], axis=0)
    alog = res.results[0]["alog"]
    anchor_vec = np.asarray(inputs["anchors"], f32)[None]
    return pred, z, anchor_vec, alog


# revision 25
# speedup vs baseline: 1.2091x; 1.1097x over previous
"""AnchorGNN forward on 8 TRN2 NeuronCores (Bass/Tile), nodes sharded 8-way.

Self-contained: hardcodes all shapes from the problem spec.
kernel(**inputs) takes the full unsharded inputs and returns
(pred [N,2], z [N,128], anchor_vec [1,A,128], anchor_logits [A,2]).

Layout strategy: activations live feature-major ([128 feat partitions, nodes
free]) so every matmul's stationary operand is a natural weight tile and
per-feature biases are per-partition scalars. The anchor-graph learner (the
dominant cost) runs the relu as a dual-op tensor_scalar on DVE (bf16, 4x
mode) and contracts on the PE with per-anchor column-masked stationary tiles
so all 32 anchors accumulate into one [32, chunk] PSUM tile. The node-dim
reductions (colsum, adj.T @ support) are computed as local partials and
combined with one tiny [32,129] AllReduce per hop; node_norm/anchor_norm are
never materialized (their scalings are factored onto the tiny side of each
product).
"""
import numpy as np

import concourse.bass as bass
import concourse.mybir as mybir
import concourse.tile as tile
from concourse.bass_utils import run_bass_kernel_spmd
from concourse.masks import make_identity

F32 = mybir.dt.float32
BF16 = mybir.dt.bfloat16
AF = mybir.ActivationFunctionType
ALU = mybir.AluOpType

N, NFEAT, NHID, A, P, HOPS = 50000, 256, 128, 32, 4, 2
NCORES = 8
NS = N // NCORES         # 6250 nodes per core
NSP = 6272               # padded to 49*128
NCH = NSP // 128         # 49 chunks of 128 nodes
EPS = 1e-12
LN_EPS = 1e-5

# matmul free-dim chunking of NSP (PSUM bank = 512 f32)
MM = []
_off = 0
for _w in [512] * 12 + [128]:
    MM.append((_off, _w))
    _off += _w
# learner super-chunks (relu granularity)
SUPS = []
_off = 0
for _w in [1024] * 6 + [128]:
    SUPS.append((_off, _w))
    _off += _w

# anchor -> (col-group, mask position) for 3-way PE column tiling
GOFF = [0, 11, 22]          # first anchor of each group
GSIZ = [11, 11, 10]
PR = 96                     # permuted anchor rows (3 groups x 32)


def j_to_gm(j):
    g = 0 if j < 11 else (1 if j < 22 else 2)
    return g, j - GOFF[g]


_MAX_WAITS = 1


_SPLIT_ENGINES = {
    mybir.EngineType.PE, mybir.EngineType.Activation, mybir.EngineType.DVE,
    mybir.EngineType.Pool, mybir.EngineType.SP,
}


def _split_waits(nc):
    """This walrus build rejects >1 sync-wait per TPB instruction; hoist extras
    onto preceding same-engine NoOps (streams are in-order, so equivalent).
    DMA-queue instructions lower via DGE rings and are left untouched."""
    ctr = [0]

    def fresh_nop(engine, waits):
        ctr[0] += 1
        nop = mybir.InstNoOp(name=f"I-waitsplit-{ctr[0]}", ins=[], outs=[])
        nop.engine = engine
        nop.sync_info = mybir.SyncInfo(on_wait=list(waits), on_update=[])
        return nop

    for fn in nc.m.functions:
        for blk in fn.blocks:
            new_insts = []
            for inst in blk.instructions:
                si = getattr(inst, "sync_info", None)
                eng = getattr(inst, "engine", None)
                if (
                    si is not None
                    and eng in _SPLIT_ENGINES
                    and si.on_wait is not None
                    and len(si.on_wait) > _MAX_WAITS
                ):
                    waits = list(si.on_wait)
                    extra, keep = waits[:-_MAX_WAITS], waits[-_MAX_WAITS:]
                    for i in range(0, len(extra), _MAX_WAITS):
                        new_insts.append(fresh_nop(eng, extra[i : i + _MAX_WAITS]))
                    inst.sync_info = mybir.SyncInfo(
                        on_wait=keep, on_update=list(si.on_update or [])
                    )
                new_insts.append(inst)
            blk.instructions[:] = new_insts
    return nc


def build_nc(split_waits=True):
    nc = bass.Bass("TRN2", target_bir_lowering=False, debug=False, num_devices=NCORES)

    def inp(name, shape):
        return nc.dram_tensor(name, shape, F32, kind="ExternalInput").ap()

    x_d = inp("x", [NSP, NFEAT])
    im_w1_d = inp("im_w1", [NFEAT, NHID])
    im_b1_d = inp("im_b1", [1, NHID])
    im_w2_d = inp("im_w2", [NHID, NHID])
    im_b2_d = inp("im_b2", [1, NHID])
    anchors_d = inp("anchors", [A, NHID])
    ml_w1_d = inp("ml_w1", [P * 2 * NHID, NHID])   # host-reshaped
    ml_b1_d = inp("ml_b1", [P, NHID])
    ml_w2_d = inp("ml_w2", [P, NHID])
    ml_b2_d = inp("ml_b2", [1, P])
    enc_w_d = inp("enc_w", [HOPS * NHID, NHID])    # host-reshaped
    enc_b_d = inp("enc_b", [HOPS, NHID])
    ln_g_d = inp("ln_g", [1, NHID])
    ln_b_d = inp("ln_b", [1, NHID])
    cls_w1_d = inp("cls_w1", [NHID, NHID])
    cls_b1_d = inp("cls_b1", [1, NHID])
    prelu_a_d = inp("prelu_a", [1, NHID])
    cls_w2_d = inp("cls_w2", [NHID, 2])
    cls_b2_d = inp("cls_b2", [1, 2])

    z_d = nc.dram_tensor("z", [NSP, NHID], F32, kind="ExternalOutput").ap()
    pred_d = nc.dram_tensor("pred", [NSP, 2], F32, kind="ExternalOutput").ap()
    alog_d = nc.dram_tensor("alog", [A, 2], F32, kind="ExternalOutput").ap()

    with tile.TileContext(nc) as tc:
        with (
            tc.tile_pool(name="wts", bufs=1) as wp,
            tc.tile_pool(name="bigs", bufs=1) as bigp,
            tc.tile_pool(name="hxp", bufs=2) as hxp,
            tc.tile_pool(name="xin", bufs=3) as xp,
            tc.tile_pool(name="work", bufs=3) as wk,
            tc.tile_pool(name="tmpl", bufs=4) as tmpl,
            tc.tile_pool(name="psmm", bufs=3, space="PSUM") as psmm,
            tc.tile_pool(name="psat", bufs=2, space="PSUM") as psat,
            tc.tile_pool(name="dram", bufs=1, space="DRAM") as drp,
        ):
            # ============ weights & small derived tiles ============
            ident = wp.tile([128, 128], F32)
            make_identity(nc, ident)
            identb = wp.tile([128, 128], BF16)
            nc.vector.tensor_copy(identb[:, :], ident[:, :])
            ones_r = wp.tile([1, 128], F32)
            nc.vector.memset(ones_r[:, :], 1.0)
            ones_rb = wp.tile([1, 128], BF16)
            nc.vector.memset(ones_rb[:, :], 1.0)

            def load_f32(name, src_ap, p, f):
                t = wp.tile([p, f], F32, name=name)
                nc.sync.dma_start(out=t[:, :], in_=src_ap)
                return t

            def cast_bf16(name, src, p, f):
                t = wp.tile([p, f], BF16, name=name)
                nc.vector.tensor_copy(t[:, :], src[:, :])
                return t

            def load_bf16(name, src_ap, p, f):
                return cast_bf16(name, load_f32(name + "_f", src_ap, p, f), p, f)

            im_w1b = [
                load_bf16(f"im_w1b{k}", im_w1_d[k * 128 : (k + 1) * 128, :], 128, NHID)
                for k in range(2)
            ]
            im_w2b = load_bf16("im_w2b", im_w2_d[:, :], 128, NHID)
            Wxb = [
                load_bf16(f"Wxb{p}", ml_w1_d[p * 256 : p * 256 + 128, :], 128, NHID)
                for p in range(P)
            ]
            Wab = [
                load_bf16(f"Wab{p}", ml_w1_d[p * 256 + 128 : p * 256 + 256, :], 128, NHID)
                for p in range(P)
            ]
            enc_wb = [
                load_bf16(f"enc_wb{i}", enc_w_d[i * 128 : (i + 1) * 128, :], 128, NHID)
                for i in range(HOPS)
            ]
            cls_w1b = load_bf16("cls_w1b", cls_w1_d[:, :], 128, NHID)
            cls_w2b = load_bf16("cls_w2b", cls_w2_d[:, :], 128, 2)

            # column-vector biases via outer product with ones ([1,1] rhs)
            def bias_col(name, src_ap, length):
                row = load_f32(name + "_r", src_ap, 1, length)
                ps = psmm.tile([length, 1], F32, name=name + "_ps", tag="mm")
                nc.tensor.matmul(ps[:, :], row[:, :], ones_r[0:1, 0:1],
                                 start=True, stop=True)
                col = wp.tile([length, 1], F32, name=name)
                nc.scalar.copy(col[:, :], ps[:, :])
                return col

            im_b1c = bias_col("im_b1c", im_b1_d[:, :], NHID)
            im_b2c = bias_col("im_b2c", im_b2_d[:, :], NHID)
            enc_bc = [
                bias_col(f"enc_bc{i}", enc_b_d[i : i + 1, :], NHID) for i in range(HOPS)
            ]
            cls_b1c = bias_col("cls_b1c", cls_b1_d[:, :], NHID)
            prelu_ac = bias_col("prelu_ac", prelu_a_d[:, :], NHID)
            cls_b2c = bias_col("cls_b2c", cls_b2_d[:, :], 2)
            neg_b1c = wp.tile([NHID, 1], F32, name="neg_b1c")
            nc.vector.tensor_scalar(out=neg_b1c[:, :], in0=cls_b1c[:, :],
                                    scalar1=-1.0, scalar2=None, op0=ALU.mult)

            # LN gamma/beta broadcast tiles [128 nodes, 128 feat]
            def bcast_row(name, src_ap):
                row = load_f32(name + "_r", src_ap, 1, NHID)
                ps = psmm.tile([128, NHID], F32, name=name + "_ps", tag="mm")
                nc.tensor.matmul(ps[:, :], ones_r[:, :], row[:, :],
                                 start=True, stop=True)
                t = wp.tile([128, NHID], BF16, name=name)
                nc.scalar.copy(t[:, :], ps[:, :])
                return t

            gBb = bcast_row("gBb", ln_g_d[:, :])
            bBb = bcast_row("bBb", ln_b_d[:, :])

            # anchorsT [128, 32] bf16
            anchors_f = load_f32("anchors_f", anchors_d[:, :], A, NHID)
            anchors_fb = cast_bf16("anchors_fb", anchors_f, A, NHID)
            aT_ps = psmm.tile([NHID, A], BF16, name="aT_ps", tag="mm")
            nc.tensor.transpose(aT_ps[:, :], anchors_fb[:, :], identb[0:A, 0:A])
            anchorsTb = wp.tile([NHID, A], BF16, name="anchorsTb")
            nc.scalar.copy(anchorsTb[:, :], aT_ps[:, :])

            # ml_b1T [128, P] f32 ; ml_w2T [128, P]
            ml_b1_f = load_f32("ml_b1_f", ml_b1_d[:, :], P, NHID)
            mb1_ps = psmm.tile([NHID, P], F32, name="mb1_ps", tag="mm")
            nc.tensor.transpose(mb1_ps[:, :], ml_b1_f[:, :], ident[0:P, 0:P])
            ml_b1T = wp.tile([NHID, P], F32, name="ml_b1T")
            nc.scalar.copy(ml_b1T[:, :], mb1_ps[:, :])

            ml_w2_f = load_f32("ml_w2_f", ml_w2_d[:, :], P, NHID)
            mw2_ps = psmm.tile([NHID, P], F32, name="mw2_ps", tag="mm")
            nc.tensor.transpose(mw2_ps[:, :], ml_w2_f[:, :], ident[0:P, 0:P])
            ml_w2Tb = wp.tile([NHID, P], BF16, name="ml_w2Tb")
            nc.scalar.copy(ml_w2Tb[:, :], mw2_ps[:, :])

            # masked stationary tiles: per (p, mask-pos m), [128, 32] with
            # w2_p in column m; shared by the 3 column-group tiles.
            w2m = wp.tile([128, P * 11 * 32], BF16, name="w2m")
            nc.vector.memset(w2m[:, :], 0.0)
            for p in range(P):
                for m in range(11):
                    cidx = (p * 11 + m) * 32 + m
                    nc.vector.tensor_copy(
                        w2m[:, cidx : cidx + 1], ml_w2Tb[:, p : p + 1]
                    )

            # b2 per-head bias columns on 32 partitions: [32, P]
            mlb2_row = load_f32("mlb2_row", ml_b2_d[:, :], 1, P)
            b2_ps = psmm.tile([128, P], F32, name="b2_ps", tag="mm")
            nc.tensor.matmul(b2_ps[:, :], ones_r[0:1, :], mlb2_row[:, :],
                             start=True, stop=True)
            b2col = wp.tile([128, P], F32, name="b2col")
            nc.scalar.copy(b2col[:, :], b2_ps[:, :])

            # perm-row validity mask [PR, 1] (1.0 on rows holding an anchor)
            selr = wp.tile([1, PR], F32, name="selr")
            nc.vector.memset(selr[:, :], 0.0)
            for g in range(3):
                nc.vector.memset(selr[0:1, 32 * g : 32 * g + GSIZ[g]], 1.0)
            selrb = wp.tile([1, PR], BF16, name="selrb")
            nc.vector.tensor_copy(selrb[:, :], selr[:, :])
            mc_ps = psmm.tile([PR, 1], F32, name="mc_ps", tag="mm")
            nc.tensor.matmul(mc_ps[:, :], selr[:, :], ones_r[0:1, 0:1],
                             start=True, stop=True)
            mask_col = wp.tile([PR, 1], F32, name="mask_col")
            nc.scalar.copy(mask_col[:, :], mc_ps[:, :])
            mrb_ps = psmm.tile([128, PR], F32, name="mrb_ps", tag="mm")
            nc.tensor.matmul(mrb_ps[:, :], ones_rb[:, :], selrb[:, :],
                             start=True, stop=True)
            maskRowB = wp.tile([128, PR], BF16, name="maskRowB")
            nc.scalar.copy(maskRowB[:, :], mrb_ps[:, :])

            # ============ input mapper: x -> hT [128, NSP] bf16 ============
            hT = bigp.tile([NHID, NSP], BF16, name="hT")
            for (off, w) in MM:
                nsub = w // 128
                xas = []
                for sNum in range(nsub):
                    c = off // 128 + sNum
                    xa = xp.tile([128, NFEAT], F32, name=f"xa{c}", tag=f"xa{sNum}")
                    nc.sync.dma_start(out=xa[:, :],
                                      in_=x_d[c * 128 : (c + 1) * 128, :])
                    xab = xp.tile([128, NFEAT], BF16, name=f"xab{c}", tag=f"xab{sNum}")
                    nc.vector.tensor_copy(xab[:, :], xa[:, :])
                    xas.append(xab)
                # transposed x: [256 feat(2 ktiles), w nodes]
                xt_ps = [
                    psmm.tile([128, w], BF16, name=f"xt_ps{off}_{k}", tag="mm")
                    for k in range(2)
                ]
                for k in range(2):
                    for sNum in range(nsub):
                        nc.tensor.transpose(
                            xt_ps[k][:, sNum * 128 : (sNum + 1) * 128],
                            xas[sNum][:, k * 128 : (k + 1) * 128],
                            identb[:, :],
                        )
                xtb = wk.tile([128, 2 * w], BF16, name=f"xtb{off}", tag="xtb")
                nc.vector.tensor_copy(xtb[:, 0:w], xt_ps[0][:, :])
                nc.vector.tensor_copy(xtb[:, w : 2 * w], xt_ps[1][:, :])
                h1_ps = psmm.tile([NHID, w], F32, name=f"h1_ps{off}", tag="mm")
                for k in range(2):
                    nc.tensor.matmul(
                        h1_ps[:, :], im_w1b[k][:, :], xtb[:, k * w : (k + 1) * w],
                        start=(k == 0), stop=(k == 1),
                    )
                r1 = wk.tile([NHID, w], BF16, name=f"r1{off}", tag="r1")
                nc.vector.tensor_scalar(
                    out=r1[:, :], in0=h1_ps[:, :], scalar1=im_b1c[:, :],
                    scalar2=0.0, op0=ALU.add, op1=ALU.max,
                )
                h2_ps = psmm.tile([NHID, w], F32, name=f"h2_ps{off}", tag="mm")
                nc.tensor.matmul(h2_ps[:, :], im_w2b[:, :], r1[:, :],
                                 start=True, stop=True)
                nc.vector.tensor_scalar(
                    out=hT[:, off : off + w], in0=h2_ps[:, :],
                    scalar1=im_b2c[:, :], scalar2=None, op0=ALU.add,
                )

            # hop-0 support depends only on hT; emit early so its PE/ACT work
            # overlaps the learner instead of the serial tail.
            def support_nm_fwd(src_T, w_b, name, s_nm=None, mm_list=None):
                if s_nm is None:
                    s_nm = bigp.tile([128, NSP], BF16, name=name, tag="s_nm")
                for (off, w) in (mm_list if mm_list is not None else MM):
                    sp_ps = psmm.tile([NHID, w], F32, name=f"{name}_ps{off}", tag="mm")
                    nc.tensor.matmul(sp_ps[:, :], w_b[:, :], src_T[:, off : off + w],
                                     start=True, stop=True)
                    sT = wk.tile([NHID, w], BF16, name=f"{name}_sT{off}", tag="sT")
                    nc.scalar.copy(sT[:, :], sp_ps[:, :])
                    tr_ps = psmm.tile([128, w], BF16, name=f"{name}_tr{off}", tag="mm")
                    nsub = w // 128
                    for sNum in range(nsub):
                        nc.tensor.transpose(
                            tr_ps[:, sNum * 128 : (sNum + 1) * 128],
                            sT[:, sNum * 128 : (sNum + 1) * 128],
                            identb[:, :],
                        )
                    nc.scalar.copy(s_nm[:, off : off + w], tr_ps[:, :])
                return s_nm

            s0_nm = bigp.tile([128, NSP], BF16, name="s0_nm", tag="s_nm")
            S0_CHUNKS = [MM[0:3], MM[3:6], MM[6:9], MM[9:13]]

            # ============ learner: adj_perm [PR, NSP] bf16 ============
            # (row 32g+m holds anchor GOFF[g]+m; other rows are sigmoid(b2))
            adjT = bigp.tile([PR, NSP], BF16, name="adjT")
            for p in range(P):
                support_nm_fwd(hT, enc_wb[0], f"s0nm{p}", s_nm=s0_nm,
                               mm_list=S0_CHUNKS[p])
                # c_pT = anchors @ Wa_p (transposed) + ml_b1[p]
                ha_ps = psmm.tile([NHID, A], F32, name=f"ha_ps{p}", tag="mm")
                nc.tensor.matmul(ha_ps[:, :], Wab[p][:, :], anchorsTb[:, :],
                                 start=True, stop=True)
                c_pT = wk.tile([NHID, A], F32, name=f"c_pT{p}", tag="cpT")
                nc.vector.tensor_scalar(
                    out=c_pT[:, :], in0=ha_ps[:, :],
                    scalar1=ml_b1T[:, p : p + 1], scalar2=None, op0=ALU.add,
                )
                # hx_p = h @ Wx_p (transposed), bf16
                hxb = hxp.tile([NHID, NSP], BF16, name=f"hxb{p}", tag="hxb")
                for (off, w) in MM:
                    hx_ps = psmm.tile([NHID, w], F32, name=f"hx_ps{p}_{off}", tag="mm")
                    nc.tensor.matmul(hx_ps[:, :], Wxb[p][:, :], hT[:, off : off + w],
                                     start=True, stop=True)
                    nc.scalar.copy(hxb[:, off : off + w], hx_ps[:, :])
                # per super-chunk: relu per anchor, masked matmuls, sigmoid
                for (soff, sw) in SUPS:
                    nbank = (sw + 511) // 512
                    pss = [
                        psat.tile([PR, min(512, sw - k * 512)], F32,
                                  name=f"at{p}_{soff}_{k}", tag=f"at{k}")
                        for k in range(nbank)
                    ]
                    jseq = []
                    for m in range(11):
                        for g in range(3):
                            if m < GSIZ[g]:
                                jseq.append((g, m, GOFF[g] + m))
                    for (g, m, j) in jseq:
                        t = tmpl.tile([128, sw], BF16, name=f"t{p}{soff}{j}", tag="t")
                        if j % 4 == 3:
                            nc.scalar.activation(
                                t[:, :], hxb[:, soff : soff + sw], AF.Relu,
                                bias=c_pT[:, j : j + 1],
                            )
                        else:
                            nc.vector.tensor_scalar(
                                out=t[:, :], in0=hxb[:, soff : soff + sw],
                                scalar1=c_pT[:, j : j + 1], scalar2=0.0,
                                op0=ALU.add, op1=ALU.max,
                            )
                        mcol = (p * 11 + m) * 32
                        for k in range(nbank):
                            kw = min(512, sw - k * 512)
                            nc.tensor.matmul(
                                pss[k][32 * g : 32 * (g + 1), :],
                                w2m[:, mcol : mcol + 32],
                                t[:, k * 512 : k * 512 + kw],
                                start=(m == 0), stop=(m == GSIZ[g] - 1),
                                tile_position=(0, 32 * g),
                                skip_group_check=True,
                            )
                    for k in range(nbank):
                        kw = min(512, sw - k * 512)
                        dst = adjT[:, soff + k * 512 : soff + k * 512 + kw]
                        if p == 0:
                            nc.scalar.activation(
                                dst, pss[k][:, :], AF.Sigmoid,
                                bias=b2col[0:PR, p : p + 1],
                            )
                        else:
                            sg = wk.tile([PR, kw], BF16, name=f"sg{p}{soff}{k}", tag="sg")
                            nc.scalar.activation(
                                sg[:, :], pss[k][:, :], AF.Sigmoid,
                                bias=b2col[0:PR, p : p + 1],
                            )
                            nc.vector.tensor_tensor(
                                out=dst, in0=dst, in1=sg[:, :], op=ALU.add
                            )

            # zero the padded node columns so node-reductions are exact
            nc.vector.memset(adjT[:, NS:NSP], 0.0)

            # ============ norms prep ============
            colsum = bigp.tile([PR, 1], F32, name="colsum")
            nc.vector.tensor_reduce(colsum[:, :], adjT[:, :],
                                    mybir.AxisListType.X, ALU.add)

            # adj_nm [128, NCH*32] bf16 : node-major chunks of adjT
            adj_nm = hxp.tile([128, NCH * PR], BF16, name="adj_nm", tag="hxb")
            for (off, w) in MM:
                nsub = w // 128
                tr_ps = psmm.tile([128, nsub * PR], BF16,
                                  name=f"adjnm_ps{off}", tag="mm")
                for s in range(nsub):
                    c = off // 128 + s
                    nc.tensor.transpose(
                        tr_ps[:, s * PR : (s + 1) * PR],
                        adjT[:, c * 128 : (c + 1) * 128],
                        identb[0:PR, 0:PR],
                    )
                nc.scalar.copy(
                    adj_nm[:, (off // 128) * PR : (off // 128 + nsub) * PR],
                    tr_ps[:, :],
                )

            # rowsum per node -> rinv broadcast tile [128, NSP] bf16
            rs_nm = bigp.tile([128, NCH], F32, name="rs_nm")
            for c in range(NCH):
                amk = wk.tile([128, PR], BF16, name=f"amk{c}", tag="amk")
                nc.vector.tensor_tensor(
                    out=amk[:, :], in0=adj_nm[:, c * PR : (c + 1) * PR],
                    in1=maskRowB[:, :], op=ALU.mult,
                )
                nc.vector.tensor_reduce(
                    rs_nm[:, c : c + 1], amk[:, :],
                    mybir.AxisListType.X, ALU.add,
                )
            nc.vector.tensor_scalar(out=rs_nm[:, :], in0=rs_nm[:, :],
                                    scalar1=EPS, scalar2=None, op0=ALU.max)
            rinv_nm = bigp.tile([128, NCH], F32, name="rinv_nm")
            nc.vector.reciprocal(rinv_nm[:, :], rs_nm[:, :])
            # rearrange to free-major [1, NSP] via per-column PE transposes
            rinv_fm = bigp.tile([1, NSP], BF16, name="rinv_fm")
            for (off, w) in MM:
                nsub = w // 128
                rt_ps = psmm.tile([1, w], F32, name=f"rt_ps{off}", tag="mm")
                for s in range(nsub):
                    c = off // 128 + s
                    nc.tensor.transpose(
                        rt_ps[:, s * 128 : (s + 1) * 128],
                        rinv_nm[:, c : c + 1], ident[:, :],
                    )
                nc.scalar.copy(rinv_fm[0:1, off : off + w], rt_ps[:, :])
            rinvB = bigp.tile([128, NSP], BF16, name="rinvB")
            for (off, w) in MM:
                nsub = w // 128
                rb_ps = psmm.tile([128, w], F32, name=f"rb_ps{off}", tag="mm")
                for s in range(nsub):
                    nc.tensor.matmul(
                        rb_ps[:, s * 128 : (s + 1) * 128],
                        ones_rb[:, :],
                        rinv_fm[0:1, off + s * 128 : off + (s + 1) * 128],
                        start=True, stop=True,
                    )
                nc.scalar.copy(rinvB[:, off : off + w], rb_ps[:, :])

            # ============ GNN hops ============
            support_nm = support_nm_fwd

            def agg_raw(s_nm, name):
                """[PR, 128] PSUM partial of adj.T @ support (over local nodes)."""
                ps = psat.tile([PR, NHID], F32, name=name, tag="at0")
                for c in range(NCH):
                    nc.tensor.matmul(
                        ps[:, :], adj_nm[:, c * PR : (c + 1) * PR],
                        s_nm[:, c * 128 : (c + 1) * 128],
                        start=(c == 0), stop=(c == NCH - 1),
                    )
                return ps

            def hop_apply(agg_b, enc_bc_i, name):
                """xc_next.T = relu((agg.T @ adjT) * rinvB + enc_b)."""
                xcT = bigp.tile([NHID, NSP], BF16, name=name)
                for (off, w) in MM:
                    xp_ps = psmm.tile([NHID, w], F32, name=f"{name}_ps{off}", tag="mm")
                    nc.tensor.matmul(xp_ps[:, :], agg_b[:, :], adjT[:, off : off + w],
                                     start=True, stop=True)
                    xpre = wk.tile([NHID, w], BF16, name=f"{name}_pre{off}", tag="sT")
                    nc.scalar.copy(xpre[:, :], xp_ps[:, :])
                    xm = wk.tile([NHID, w], BF16, name=f"{name}_m{off}", tag="xm")
                    nc.vector.tensor_tensor(
                        out=xm[:, :], in0=xpre[:, :],
                        in1=rinvB[:, off : off + w], op=ALU.mult,
                    )
                    nc.vector.tensor_scalar(
                        out=xcT[:, off : off + w], in0=xm[:, :],
                        scalar1=enc_bc_i[:, :], scalar2=0.0,
                        op0=ALU.add, op1=ALU.max,
                    )
                return xcT

            # hop 0: fused AllReduce of [agg0_raw | colsum]
            agg0_ps = agg_raw(s0_nm, "agg0_ps")
            arbuf = bigp.tile([PR, 132], F32, name="arbuf")
            nc.scalar.copy(arbuf[:, 0:NHID], agg0_ps[:, :])
            nc.vector.tensor_copy(arbuf[:, NHID : NHID + 1], colsum[:, :])
            drin0 = drp.tile([PR, 129], F32, name="drin0")
            drout0 = drp.tile([PR, 129], F32, name="drout0")
            nc.sync.dma_start(out=drin0[:, :], in_=arbuf[:, 0:129])
            nc.gpsimd.collective_compute(
                "AllReduce", ALU.add,
                replica_groups=[list(range(NCORES))],
                ins=[drin0[:, :].opt()], outs=[drout0[:, :].opt()],
            )
            arres0 = bigp.tile([PR, 132], F32, name="arres0")
            nc.sync.dma_start(out=arres0[:, 0:129], in_=drout0[:, :])

            csc = bigp.tile([PR, 1], F32, name="csc")
            nc.vector.tensor_scalar(out=csc[:, :], in0=arres0[:, NHID : NHID + 1],
                                    scalar1=EPS, scalar2=None, op0=ALU.max)
            csinv = bigp.tile([PR, 1], F32, name="csinv")
            nc.vector.reciprocal(csinv[:, :], csc[:, :])
            # fold the perm-row validity mask into the scale
            nc.vector.tensor_scalar(out=csinv[:, :], in0=csinv[:, :],
                                    scalar1=mask_col[:, :], scalar2=None,
                                    op0=ALU.mult)
            agg0b = bigp.tile([PR, NHID], BF16, name="agg0b")
            nc.vector.tensor_scalar(out=agg0b[:, :], in0=arres0[:, 0:NHID],
                                    scalar1=csinv[:, :], scalar2=None, op0=ALU.mult)

            xc1T = hop_apply(agg0b, enc_bc[0], "xc1T")

            # hop 1
            s1_nm = support_nm(xc1T, enc_wb[1], "s1_nm")
            agg1_ps = agg_raw(s1_nm, "agg1_ps")
            arbuf1 = bigp.tile([PR, NHID], F32, name="arbuf1")
            nc.scalar.copy(arbuf1[:, :], agg1_ps[:, :])
            drin1 = drp.tile([PR, NHID], F32, name="drin1")
            drout1 = drp.tile([PR, NHID], F32, name="drout1")
            nc.sync.dma_start(out=drin1[:, :], in_=arbuf1[:, :])
            nc.gpsimd.collective_compute(
                "AllReduce", ALU.add,
                replica_groups=[list(range(NCORES))],
                ins=[drin1[:, :].opt()], outs=[drout1[:, :].opt()],
            )
            arres1 = bigp.tile([PR, NHID], F32, name="arres1")
            nc.sync.dma_start(out=arres1[:, :], in_=drout1[:, :])
            agg1b = bigp.tile([PR, NHID], BF16, name="agg1b")
            nc.vector.tensor_scalar(out=agg1b[:, :], in0=arres1[:, :],
                                    scalar1=csinv[:, :], scalar2=None, op0=ALU.mult)

            xc2T = hop_apply(agg1b, enc_bc[1], "xc2T")

            # ============ residual + LN (node-major) ============
            zTb = bigp.tile([NHID, NSP], BF16, name="zTb", tag="xc1T")
            for (off, w) in MM:
                nc.vector.tensor_tensor(
                    out=zTb[:, off : off + w], in0=xc2T[:, off : off + w],
                    in1=hT[:, off : off + w], op=ALU.add,
                )
            z_nm = bigp.tile([128, NSP], BF16, name="z_nm", tag="s_nm")
            for (off, w) in MM:
                tr_ps = psmm.tile([128, w], BF16, name=f"znm_ps{off}", tag="mm")
                nsub = w // 128
                for s in range(nsub):
                    nc.tensor.transpose(
                        tr_ps[:, s * 128 : (s + 1) * 128],
                        zTb[:, off + s * 128 : off + (s + 1) * 128],
                        identb[:, :],
                    )
                nc.scalar.copy(z_nm[:, off : off + w], tr_ps[:, :])

            mu_all = bigp.tile([128, NCH], F32, name="mu_all")
            sq_all = bigp.tile([128, NCH], F32, name="sq_all")
            for (off, w) in MM:
                nsub = w // 128
                c0 = off // 128
                zc3 = z_nm[:, off : off + w].rearrange("p (c f) -> p c f", f=128)
                nc.vector.tensor_reduce(mu_all[:, c0 : c0 + nsub], zc3,
                                        mybir.AxisListType.X, ALU.add)
                sqt = wk.tile([128, w], BF16, name=f"sqt{off}", tag="sqt", bufs=1)
                nc.vector.tensor_tensor(out=sqt[:, :], in0=z_nm[:, off : off + w],
                                        in1=z_nm[:, off : off + w], op=ALU.mult)
                nc.vector.tensor_reduce(
                    sq_all[:, c0 : c0 + nsub],
                    sqt[:, :].rearrange("p (c f) -> p c f", f=128),
                    mybir.AxisListType.X, ALU.add)
            nc.vector.tensor_scalar(out=mu_all[:, :], in0=mu_all[:, :],
                                    scalar1=1.0 / NHID, scalar2=None, op0=ALU.mult)
            nc.vector.tensor_scalar(out=sq_all[:, :], in0=sq_all[:, :],
                                    scalar1=1.0 / NHID, scalar2=None, op0=ALU.mult)
            msq = bigp.tile([128, NCH], F32, name="msq")
            nc.vector.tensor_tensor(out=msq[:, :], in0=mu_all[:, :],
                                    in1=mu_all[:, :], op=ALU.mult)
            var = bigp.tile([128, NCH], F32, name="var")
            nc.vector.tensor_tensor(out=var[:, :], in0=sq_all[:, :],
                                    in1=msq[:, :], op=ALU.subtract)
            stdt = bigp.tile([128, NCH], F32, name="stdt")
            lneps_c = wp.tile([128, 1], F32, name="lneps_c")
            nc.vector.memset(lneps_c[:, :], LN_EPS)
            nc.scalar.activation(stdt[:, :], var[:, :], AF.Sqrt, bias=lneps_c[:, :])
            rstd = bigp.tile([128, NCH], F32, name="rstd")
            nc.vector.reciprocal(rstd[:, :], stdt[:, :])

            znT = bigp.tile([NHID, NSP], BF16, name="znT", tag="rinvB")
            for (off, w) in MM:
                nsub = w // 128
                trz_ps = psmm.tile([128, w], F32, name=f"znt_ps{off}", tag="mm")
                for s in range(nsub):
                    c = off // 128 + s
                    zc = z_nm[:, c * 128 : (c + 1) * 128]
                    t1 = wk.tile([128, 128], BF16, name=f"lnt1_{c}", tag="lnt1")
                    nc.vector.tensor_scalar(
                        out=t1[:, :], in0=zc,
                        scalar1=mu_all[:, c : c + 1], scalar2=rstd[:, c : c + 1],
                        op0=ALU.subtract, op1=ALU.mult,
                    )
                    t2 = wk.tile([128, 128], BF16, name=f"lnt2_{c}", tag="lnt2")
                    nc.vector.tensor_tensor(out=t2[:, :], in0=t1[:, :],
                                            in1=gBb[:, :], op=ALU.mult)
                    znf = wk.tile([128, 128], F32, name=f"znf_{c}", tag="znf")
                    nc.vector.tensor_tensor(out=znf[:, :], in0=t2[:, :],
                                            in1=bBb[:, :], op=ALU.add)
                    if c * 128 < NS:
                        hi = min(128, NS - c * 128)
                        nc.sync.dma_start(
                            out=z_d[c * 128 : c * 128 + hi, :], in_=znf[0:hi, :]
                        )
                    nc.tensor.transpose(
                        trz_ps[:, s * 128 : (s + 1) * 128], znf[:, :], ident[:, :]
                    )
                if True:
                    nc.scalar.copy(znT[:, off : off + w], trz_ps[:, :])

            # ============ classifier ============
            for (off, w) in MM:
                t1_ps = psmm.tile([NHID, w], F32, name=f"cls_ps{off}", tag="mm")
                nc.tensor.matmul(t1_ps[:, :], cls_w1b[:, :], znT[:, off : off + w],
                                 start=True, stop=True)
                r1p = wk.tile([NHID, w], BF16, name=f"r1p{off}", tag="sT")
                nc.scalar.activation(r1p[:, :], t1_ps[:, :], AF.Relu,
                                     bias=cls_b1c[:, :])
                r2p = wk.tile([NHID, w], BF16, name=f"r2p{off}", tag="xm")
                nc.scalar.activation(r2p[:, :], t1_ps[:, :], AF.Relu,
                                     bias=neg_b1c[:, :], scale=-1.0)
                r2s = wk.tile([NHID, w], BF16, name=f"r2s{off}", tag="r2s")
                nc.vector.tensor_scalar(out=r2s[:, :], in0=r2p[:, :],
                                        scalar1=prelu_ac[:, :], scalar2=None,
                                        op0=ALU.mult)
                t1b = wk.tile([NHID, w], BF16, name=f"t1b{off}", tag="t1b")
                nc.vector.tensor_tensor(out=t1b[:, :], in0=r1p[:, :],
                                        in1=r2s[:, :], op=ALU.subtract)
                pr_ps = psmm.tile([2, w], F32, name=f"pr_ps{off}", tag="mm")
                nc.tensor.matmul(pr_ps[:, :], cls_w2b[:, :], t1b[:, :],
                                 start=True, stop=True)
                predc = wk.tile([2, w], F32, name=f"predc{off}", tag="predc")
                nc.scalar.activation(predc[:, :], pr_ps[:, :], AF.Identity,
                                     bias=cls_b2c[:, :])
                nsub = w // 128
                pt_ps = psmm.tile([128, nsub * 2], F32, name=f"pt_ps{off}", tag="mm")
                for s in range(nsub):
                    nc.tensor.transpose(
                        pt_ps[:, s * 2 : (s + 1) * 2],
                        predc[:, s * 128 : (s + 1) * 128],
                        ident[0:2, 0:2],
                    )
                predn = wk.tile([128, nsub * 2], F32, name=f"predn{off}", tag="predn")
                nc.scalar.copy(predn[:, :], pt_ps[:, :])
                for s in range(nsub):
                    c = off // 128 + s
                    if c * 128 < NS:
                        hi = min(128, NS - c * 128)
                        nc.sync.dma_start(
                            out=pred_d[c * 128 : c * 128 + hi, :],
                            in_=predn[0:hi, s * 2 : (s + 1) * 2],
                        )

            # anchors through the classifier
            a1_ps = psmm.tile([NHID, A], F32, name="a1_ps", tag="mm")
            nc.tensor.matmul(a1_ps[:, :], cls_w1b[:, :], anchorsTb[:, :],
                             start=True, stop=True)
            ar1 = wk.tile([NHID, A], BF16, name="ar1", tag="cpT")
            nc.scalar.activation(ar1[:, :], a1_ps[:, :], AF.Relu,
                                 bias=cls_b1c[:, :])
            ar2 = wk.tile([NHID, A], BF16, name="ar2", tag="ar2")
            nc.scalar.activation(ar2[:, :], a1_ps[:, :], AF.Relu,
                                 bias=neg_b1c[:, :], scale=-1.0)
            ar2s = wk.tile([NHID, A], BF16, name="ar2s", tag="ar2s")
            nc.vector.tensor_scalar(out=ar2s[:, :], in0=ar2[:, :],
                                    scalar1=prelu_ac[:, :], scalar2=None,
                                    op0=ALU.mult)
            a1b = wk.tile([NHID, A], BF16, name="a1b", tag="a1b")
            nc.vector.tensor_tensor(out=a1b[:, :], in0=ar1[:, :],
                                    in1=ar2s[:, :], op=ALU.subtract)
            a2_ps = psmm.tile([2, A], F32, name="a2_ps", tag="mm")
            nc.tensor.matmul(a2_ps[:, :], cls_w2b[:, :], a1b[:, :],
                             start=True, stop=True)
            alsb = wk.tile([2, A], F32, name="alsb", tag="predc")
            nc.scalar.activation(alsb[:, :], a2_ps[:, :], AF.Identity,
                                 bias=cls_b2c[:, :])
            alt_ps = psmm.tile([A, 2], F32, name="alt_ps", tag="mm")
            nc.tensor.transpose(alt_ps[:, :], alsb[:, :], ident[0:2, 0:2])
            alog_sb = wk.tile([A, 2], F32, name="alog_sb", tag="alog")
            nc.scalar.copy(alog_sb[:, :], alt_ps[:, :])
            nc.sync.dma_start(out=alog_d[:, :], in_=alog_sb[:, :])

    if split_waits:
        _split_waits(nc)
    return nc


_NC = None


def kernel(**inputs):
    global _NC
    if _NC is None:
        _NC = build_nc()

    f32 = np.float32
    x = np.asarray(inputs["x"], f32)
    base = {
        "im_w1": np.asarray(inputs["im_w1"], f32),
        "im_b1": np.asarray(inputs["im_b1"], f32).reshape(1, NHID),
        "im_w2": np.asarray(inputs["im_w2"], f32),
        "im_b2": np.asarray(inputs["im_b2"], f32).reshape(1, NHID),
        "anchors": np.asarray(inputs["anchors"], f32),
        "ml_w1": np.asarray(inputs["ml_w1"], f32).reshape(P * 2 * NHID, NHID),
        "ml_b1": np.asarray(inputs["ml_b1"], f32),
        "ml_w2": np.asarray(inputs["ml_w2"], f32),
        "ml_b2": np.asarray(inputs["ml_b2"], f32).reshape(1, P),
        "enc_w": np.asarray(inputs["enc_w"], f32).reshape(HOPS * NHID, NHID),
        "enc_b": np.asarray(inputs["enc_b"], f32),
        "ln_g": np.asarray(inputs["ln_g"], f32).reshape(1, NHID),
        "ln_b": np.asarray(inputs["ln_b"], f32).reshape(1, NHID),
        "cls_w1": np.asarray(inputs["cls_w1"], f32),
        "cls_b1": np.asarray(inputs["cls_b1"], f32).reshape(1, NHID),
        "prelu_a": np.asarray(inputs["prelu_a"], f32).reshape(1, NHID),
        "cls_w2": np.asarray(inputs["cls_w2"], f32),
        "cls_b2": np.asarray(inputs["cls_b2"], f32).reshape(1, 2),
    }
    in_maps = []
    for i in range(NCORES):
        xs = x[i * NS : (i + 1) * NS]
        xs = np.pad(xs, ((0, NSP - NS), (0, 0)))
        m = dict(base)
        m["x"] = np.ascontiguousarray(xs)
        in_maps.append(m)

    res = run_bass_kernel_spmd(_NC, in_maps, core_ids=list(range(NCORES)))
    pred = np.concatenate([res.results[i]["pred"][:NS] for i in range(NCORES)], axis=0)
    z = np.concatenate([res.results[i]["z"][:NS] for i in range(NCORES)], axis=0)
    alog = res.results[0]["alog"]
    anchor_vec = np.asarray(inputs["anchors"], f32)[None]
    return pred, z, anchor_vec, alog
